# revision 8
# baseline (speedup 1.0000x reference)
"""GCN message-passing kernel for 8 Trainium2 NeuronCores.

Strategy: shard destination nodes across cores (6272 rows/core). Each core
aggregates all edges targeting its rows by gathering source rows from a
replicated bf16 node-feature table with SWDGE dma_gather in PREPARE_ONLY +
trigger_dma mode (descriptor generation pipelines with the DMA transfers
instead of blocking GpSimd for the full round-trip) and contracting each
128-edge chunk against a one-hot selector generated ON-CHIP by one fused DVE
tensor_scalar op per chunk: sel = (iota == dstrel) * norm, with dstrel/norm
streamed as tiny [128, C] scalar tables. The aggregation runs transposed
(psum[feat, dst]) so bias+relu+row-sum fuse into one Activation-engine op.
Layer tables are computed node-major directly (lhsT = xT tile, rhs = W; no PE
transpose) and ping-pong between two DRAM buffers so each AllGather overlaps
the previous layer's aggregation. SE attention + 1x1 conv are tiny and
replicated; the final output is produced transposed and fixed up on host.
"""
import os
import sys

sys.path.insert(0, "/opt/trn_rl_repo")

from contextlib import ExitStack

import ml_dtypes
import numpy as np

import concourse.bacc as bacc
import concourse.tile as tile
from concourse.tile import add_dep_helper
from concourse import bass_isa, mybir
from concourse.bass_utils import run_bass_kernel_spmd

N = 50000
FM = 128
E = 800000
NCORES = 8
NPOS = 49                  # 128-row tiles per core
RPC = NPOS * 128           # 6272 rows per core
NPAD = NCORES * RPC        # 50176
HALF = NPAD // 2           # 25088 (int16 gather index limit per table half)
SG = 4                     # positions per gather supergroup
VIEWS = ("f", "s", "g")
LAYERS = [("f", 1), ("s", 1), ("g", 1), ("f", 2), ("s", 2), ("g", 2)]

f32 = mybir.dt.float32
bf16 = mybir.dt.bfloat16
i16 = mybir.dt.int16
bfnp = ml_dtypes.bfloat16

_last_exec_time_ns = None


def _split_multiwaits(nc):
    """This walrus build accepts only ONE sync-wait per instruction; split
    extras into preceding same-engine single-wait NoOps (sequencer executes
    waits in program order, so semantics are preserved)."""
    n = 0
    for fn in nc.m.functions:
        for bb in fn.blocks:
            newlist = []
            for inst in bb.instructions:
                si = inst.sync_info
                if si is not None and len(si.on_wait) > 1:
                    waits = list(si.on_wait)
                    for w in waits[:-1]:
                        nop = mybir.InstNoOp(name=f"WSPL-{nc.next_id()}", ins=[], outs=[])
                        nop.engine = inst.engine
                        nop.sync_info = mybir.SyncInfo(on_wait=[w], on_update=[])
                        newlist.append(nop)
                        n += 1
                    si.on_wait = [waits[-1]]
                newlist.append(inst)
            bb.instructions = newlist
    return n


def _prep_view(edges, ew):
    """Host edge preprocessing for one view: append self-loops, compute the
    symmetric GCN normalization, shard by destination across cores, group by
    (dst tile, src half), pad runs to 128-edge chunks (uniform across cores).

    Returns (idx_arrs, dr_arrs, nm_arrs, NLO, NHI): per-core SWDGE index
    arrays plus per-edge-slot dstrel/norm scalar tables ([128, C] bf16) from
    which the device generates one-hot selector chunks on-chip."""
    src = np.concatenate([edges[0], np.arange(N, dtype=np.int64)])
    dst = np.concatenate([edges[1], np.arange(N, dtype=np.int64)])
    w = np.concatenate([ew.astype(np.float64), np.ones(N)])
    deg = np.bincount(dst, weights=w, minlength=N)
    dis = 1.0 / np.sqrt(deg)
    norm = (dis[src] * w * dis[dst]).astype(np.float32)

    core = dst // RPC
    pos = (dst % RPC) // 128
    dstrel = (dst % 128).astype(np.int64)
    half = (src >= HALF).astype(np.int64)
    idx = (src - HALF * half).astype(np.int16)

    # counts[c, p, h]
    key = (core * NPOS + pos) * 2 + half
    counts = np.bincount(key, minlength=NCORES * NPOS * 2).reshape(NCORES, NPOS, 2)
    chunks = -(-counts // 128)  # ceil
    NLO = chunks[:, :, 0].max(axis=0)
    NHI = chunks[:, :, 1].max(axis=0)

    order = np.lexsort((half, pos, core))
    norm_s, dstrel_s, idx_s, key_s = (
        norm[order], dstrel[order], idx[order], key[order])
    starts = np.searchsorted(key_s, np.arange(NCORES * NPOS * 2))
    ends = np.searchsorted(key_s, np.arange(NCORES * NPOS * 2), side="right")

    C = int((NLO + NHI).sum())
    idx_arrs, dr_arrs, nm_arrs = [], [], []
    sgs = [list(range(s, min(s + SG, NPOS))) for s in range(0, NPOS, SG)]
    for c in range(NCORES):
        idx_a = np.zeros(C * 128, np.int16)
        dr_a = np.zeros(C * 128, np.int64)
        w_a = np.zeros(C * 128, np.float32)
        off = 0
        for sg in sgs:
            for h in range(2):
                for p in sg:
                    k = (c * NPOS + p) * 2 + h
                    s0, e0 = starts[k], ends[k]
                    n = e0 - s0
                    nch = (NLO if h == 0 else NHI)[p]
                    idx_a[off:off + n] = idx_s[s0:e0]
                    dr_a[off:off + n] = dstrel_s[s0:e0]
                    w_a[off:off + n] = norm_s[s0:e0]
                    off += nch * 128
        assert off == C * 128
        # device layouts
        idx_wrapped = np.tile(idx_a.reshape(-1, 16).T, (8, 1)).copy()  # [128, C*8]
        dr_dev = np.ascontiguousarray(dr_a.reshape(C, 128).T).astype(bfnp)
        nm_dev = np.ascontiguousarray(w_a.reshape(C, 128).T).astype(bfnp)
        idx_arrs.append(idx_wrapped)
        dr_arrs.append(dr_dev)
        nm_arrs.append(nm_dev)
    return idx_arrs, dr_arrs, nm_arrs, NLO.astype(int), NHI.astype(int)


def _build(meta):
    """Build the SPMD program. meta[v] = (NLO, NHI, C) per view."""
    nc = bacc.Bacc("TRN2", target_bir_lowering=False, debug=False,
                   num_devices=NCORES,
                   dynamic_dma_scratch_size=int(os.environ.get("KERNEL_DMA_SCRATCH", "16384")))

    # ---- I/O ----
    xT_in = nc.dram_tensor("xT_slice", [128, RPC], bf16, kind="ExternalInput").ap()
    W_in, b_in = {}, {}
    for nm in ["f1", "f2", "s1", "s2", "g1", "g2"]:
        W_in[nm] = nc.dram_tensor(f"W_{nm}", [FM, FM], bf16, kind="ExternalInput").ap()
        b_in[nm] = nc.dram_tensor(f"b_{nm}", [FM], f32, kind="ExternalInput").ap()
    idx_in, dr_in, nm_in = {}, {}, {}
    for v in VIEWS:
        C = meta[v][2]
        idx_in[v] = nc.dram_tensor(f"idx_{v}", [128, C * 8], i16, kind="ExternalInput").ap()
        dr_in[v] = nc.dram_tensor(f"dr_{v}", [128, C], bf16, kind="ExternalInput").ap()
        nm_in[v] = nc.dram_tensor(f"nm_{v}", [128, C], bf16, kind="ExternalInput").ap()
    iota_in = nc.dram_tensor("iota", [128, 128], bf16, kind="ExternalInput").ap()
    fc1wT_in = nc.dram_tensor("fc1wT", [6, 30], f32, kind="ExternalInput").ap()
    fc1b_in = nc.dram_tensor("fc1b", [30], f32, kind="ExternalInput").ap()
    fc2wT_in = nc.dram_tensor("fc2wT", [30, 6], f32, kind="ExternalInput").ap()
    fc2b_in = nc.dram_tensor("fc2b", [6], f32, kind="ExternalInput").ap()
    cnnw_in = nc.dram_tensor("cnnw", [6], f32, kind="ExternalInput").ap()
    cnnb_in = nc.dram_tensor("cnnb", [1], f32, kind="ExternalInput").ap()
    corr_in = nc.dram_tensor("corr", [6], f32, kind="ExternalInput").ap()
    out_d = nc.dram_tensor("out_slice", [RPC, FM], f32, kind="ExternalOutput").ap()

    dma_sem = nc.alloc_semaphore("gather_dma")
    NGSEM = 4
    gsems = [nc.alloc_semaphore(f"gsem{i}") for i in range(NGSEM)]

    with tile.TileContext(nc) as tc, ExitStack() as ctx:
        singles = ctx.enter_context(tc.tile_pool(name="singles", bufs=1))
        pool = ctx.enter_context(tc.tile_pool(name="pool", bufs=3))
        selp = ctx.enter_context(tc.tile_pool(name="selp", bufs=2))
        gpo = ctx.enter_context(tc.tile_pool(name="gpo", bufs=2))
        fpo = ctx.enter_context(tc.tile_pool(name="fpo", bufs=4))
        psA = ctx.enter_context(tc.tile_pool(name="psA", bufs=4, space="PSUM"))
        psB = ctx.enter_context(tc.tile_pool(name="psB", bufs=2, space="PSUM"))
        dram = ctx.enter_context(tc.tile_pool(name="dram", bufs=1, space="DRAM"))

        tables = [dram.tile([NPAD, FM], bf16, name=f"table{i}") for i in range(2)]
        tab_slices = [dram.tile([RPC, FM], bf16, name=f"tab_slice{i}") for i in range(2)]
        fT_sl = [dram.tile([RPC, 128], bf16, name=f"fT_sl{i}") for i in range(6)]
        pool6_in = dram.tile([6, 1], f32, name="pool6_in")
        pool6_out = dram.tile([6, 1], f32, name="pool6_out")
        a_scr = dram.tile([6], f32, name="a_scr")

        # ---- constants ----
        iota_sb = singles.tile([128, 128], bf16)
        nc.sync.dma_start(out=iota_sb[:], in_=iota_in[:])
        W_sb, bb_bc = {}, {}
        for nm in ["f1", "f2", "s1", "s2", "g1", "g2"]:
            W_sb[nm] = singles.tile([FM, FM], bf16, tag=f"W_{nm}", name=f"Wsb_{nm}")
            nc.sync.dma_start(out=W_sb[nm][:], in_=W_in[nm][:])
            bb_bc[nm] = singles.tile([128, FM], f32, tag=f"bb_{nm}", name=f"bbsb_{nm}")
            nc.gpsimd.dma_start(out=bb_bc[nm][:], in_=b_in[nm].partition_broadcast(128))
        pooled_acc = singles.tile([128, 6], f32)
        nc.vector.memset(pooled_acc[:], 0.0)

        idx_sb, dr_sb, nm_sb = {}, {}, {}
        for v in VIEWS:
            C = meta[v][2]
            idx_sb[v] = singles.tile([128, C * 8], i16, tag=f"idx_{v}", name=f"idxsb_{v}")
            nc.sync.dma_start(out=idx_sb[v][:], in_=idx_in[v][:])
            dr_sb[v] = singles.tile([128, C], bf16, tag=f"dr_{v}", name=f"drsb_{v}")
            nc.sync.dma_start(out=dr_sb[v][:], in_=dr_in[v][:])
            nm_sb[v] = singles.tile([128, C], bf16, tag=f"nm_{v}", name=f"nmsb_{v}")
            nc.sync.dma_start(out=nm_sb[v][:], in_=nm_in[v][:])

        # per-table-buffer state for manual collective/gather dep tracking
        # (custom-DMA APs over DRAM pool tiles are not reliably dep-tracked)
        tabst = [{"ag": None, "preps": []} for _ in range(2)]
        slice_ag = [None, None]   # last AllGather reading tab_slices[i]
        gstate = {"sg": 0, "cum": [0] * NGSEM}

        def tab_phase(src_kind, vsrc_l, Wn, ts):
            """tab_slices[ts] = cast_bf16(src @ W) for own rows, node-major.
            src 'x': xT input; src 'f': fT_sl[vsrc_l] (both [feat, node])."""
            war = slice_ag[ts]
            for p in range(NPOS):
                cols = slice(p * 128, (p + 1) * 128)
                t_fn = pool.tile([128, 128], bf16, tag="tabin")
                if src_kind == "x":
                    ld = nc.sync.dma_start(out=t_fn[:], in_=xT_in[:, cols])
                else:
                    ld = nc.sync.dma_start_transpose(
                        out=t_fn[:], in_=fT_sl[vsrc_l][p * 128:(p + 1) * 128, :])
                pm = psB.tile([128, 128], f32, tag="tabps")
                nc.tensor.matmul(pm[:], lhsT=t_fn[:], rhs=W_sb[Wn][:], start=True, stop=True)
                tb = pool.tile([128, 128], bf16, tag="tbf")
                nc.scalar.copy(out=tb[:], in_=pm[:])
                st = nc.sync.dma_start(out=tab_slices[ts][p * 128:(p + 1) * 128, :], in_=tb[:])
                if war is not None:
                    add_dep_helper(st.ins, war.ins, reason="tab_slice WAR")
                yield st

        def allgather_table(tab_stores, ts, buf):
            ag = nc.gpsimd.collective_compute(
                "AllGather", mybir.AluOpType.bypass,
                replica_groups=[list(range(NCORES))],
                ins=[tab_slices[ts][:]], outs=[tables[buf][:]],
            )
            for st in tab_stores:
                add_dep_helper(ag.ins, st.ins, reason="tab_slice RAW")
            for g in tabst[buf]["preps"]:
                add_dep_helper(ag.ins, g.ins, reason="table WAR")
            # WAR vs in-flight prepared gathers: prep engine-ticks only cover
            # descriptor GENERATION; wait for the transfers via the gather
            # sems, attached directly to the collective so the scheduler
            # cannot float the wait away from it.
            for i in range(NGSEM):
                if gstate["cum"][i] > 0:
                    ag.wait_op(gsems[i], gstate["cum"][i], "sem-ge", check=False)
            tabst[buf] = {"ag": ag, "preps": []}
            slice_ag[ts] = ag

        def agg_phase(v, Wn, l_out, buf):
            NLO, NHI, C = meta[v]
            ag = tabst[buf]["ag"]
            sgs = [list(range(s, min(s + SG, NPOS))) for s in range(0, NPOS, SG)]
            chunk_base = 0  # global chunk counter
            idx_col = 0     # column offset into idx_sb (units of 16 idxs)
            for sg in sgs:
                nlo = int(sum(NLO[p] for p in sg))
                nhi = int(sum(NHI[p] for p in sg))
                nch_sg = nlo + nhi
                # generate this supergroup's selector chunks on-chip in two
                # batched DVE ops over broadcast APs:
                # sel[e, c, d] = (iota[e, d] == dr[e, c]) * nm[e, c]
                nch_c = max(nch_sg, 1)
                selsb = selp.tile([128, nch_c, 128], bf16, tag="sel")
                iota3 = iota_sb[:].unsqueeze(1).broadcast_to([128, nch_c, 128])
                dr3 = dr_sb[v][:, chunk_base:chunk_base + nch_c].unsqueeze(2) \
                    .broadcast_to([128, nch_c, 128])
                nm3 = nm_sb[v][:, chunk_base:chunk_base + nch_c].unsqueeze(2) \
                    .broadcast_to([128, nch_c, 128])
                nc.vector.tensor_tensor(out=selsb[:], in0=iota3, in1=dr3,
                                        op=mybir.AluOpType.is_equal)
                nc.vector.tensor_tensor(out=selsb[:], in0=selsb[:], in1=nm3,
                                        op=mybir.AluOpType.mult)
                glo = gpo.tile([128, max(nlo, 1), 128], bf16, tag="glo")
                ghi = gpo.tile([128, max(nhi, 1), 128], bf16, tag="ghi")
                sync_gather = os.environ.get("KERNEL_SYNC_GATHER", "0") == "1"
                GMAXC = int(os.environ.get("KERNEL_GMAXC", "8"))
                si = gstate["sg"] % NGSEM
                gstate["sg"] += 1
                nprep_sg = 0
                for half_i, (nh, gt, lohi) in enumerate(
                        (((nlo, glo, (0, HALF)), (nhi, ghi, (HALF, NPAD))))):
                    for g0 in range(0, nh, GMAXC):
                        gn = min(GMAXC, nh - g0)
                        if sync_gather:
                            gi = nc.gpsimd.dma_gather(
                                out_ap=gt[:, g0:g0 + gn, :],
                                in_ap=tables[buf][lohi[0]:lohi[1], :],
                                idxs_ap=idx_sb[v][:, idx_col:idx_col + gn * 8],
                                num_idxs=gn * 128, num_idxs_reg=gn * 128, elem_size=128,
                            )
                            add_dep_helper(gi.ins, ag.ins, reason="table RAW")
                            tabst[buf]["preps"].append(gi)
                        else:
                            gi = nc.gpsimd.dma_gather(
                                out_ap=gt[:, g0:g0 + gn, :],
                                in_ap=tables[buf][lohi[0]:lohi[1], :],
                                idxs_ap=idx_sb[v][:, idx_col:idx_col + gn * 8],
                                num_idxs=gn * 128, num_idxs_reg=gn * 128, elem_size=128,
                                prepare_only=True, sem=gsems[si],
                            )
                            add_dep_helper(gi.ins, ag.ins, reason="table RAW")
                            tr = nc.gpsimd.trigger_dma(count=None)
                            add_dep_helper(tr.ins, ag.ins, reason="table RAW @trigger")
                            nprep_sg += 1
                        idx_col += gn * 8
                if not sync_gather and nprep_sg:
                    # Tile's gen_mode==1 DMASW lane protocol releases consumers
                    # one prep early; gate this supergroup's matmuls on the
                    # explicit per-supergroup DMA-completion sem instead (the
                    # wait is attached to the first matmul of each position).
                    gstate["cum"][si] += 16 * nprep_sg
                # chunk order in sel array: [lo(p0)..lo(pk)] then [hi(p0)..hi(pk)]
                lo_off, off = {}, 0
                for p in sg:
                    lo_off[p] = off
                    off += int(NLO[p])
                hi_off, off = {}, 0
                for p in sg:
                    hi_off[p] = off
                    off += int(NHI[p])
                for p in sg:
                    # psum[dst, feat]: lhsT = one-hot selector (stationary; its
                    # DVE producer deps land on the Ldweights), rhs = gathered
                    # rows (moving; rhs deps land on the matmul itself, where
                    # the explicit gather-completion wait is attached).
                    ps = psA.tile([128, 128], f32, tag="agg")
                    nch = int(NLO[p] + NHI[p])
                    ci = 0
                    for k in range(int(NLO[p])):
                        cg = lo_off[p] + k  # sel col block within supergroup
                        mm = nc.tensor.matmul(ps[:], rhs=glo[:, lo_off[p] + k, :],
                                              lhsT=selsb[:, cg, :],
                                              start=(ci == 0), stop=(ci == nch - 1))
                        if ci == 0 and not sync_gather and gstate["cum"][si] > 0:
                            mm.wait_op(gsems[si], gstate["cum"][si], "sem-ge", check=False)
                        ci += 1
                    for k in range(int(NHI[p])):
                        cg = nlo + hi_off[p] + k
                        mm = nc.tensor.matmul(ps[:], rhs=ghi[:, hi_off[p] + k, :],
                                              lhsT=selsb[:, cg, :],
                                              start=(ci == 0), stop=(ci == nch - 1))
                        if ci == 0 and not sync_gather and gstate["cum"][si] > 0:
                            mm.wait_op(gsems[si], gstate["cum"][si], "sem-ge", check=False)
                        ci += 1
                    # postprocess: fT = relu(agg + b), fused row-sum for pooling
                    tadd = fpo.tile([128, 128], f32, tag="tadd")
                    nc.vector.tensor_tensor(out=tadd[:], in0=ps[:], in1=bb_bc[Wn][:],
                                            op=mybir.AluOpType.add)
                    ft = fpo.tile([128, 128], bf16, tag="ftile")
                    racc = fpo.tile([128, 1], f32, tag="racc")
                    nc.scalar.activation(
                        out=ft[:], in_=tadd[:], func=mybir.ActivationFunctionType.Relu,
                        accum_out=racc[:])
                    nc.vector.tensor_tensor(out=pooled_acc[:, l_out:l_out + 1],
                                            in0=pooled_acc[:, l_out:l_out + 1],
                                            in1=racc[:], op=mybir.AluOpType.add)
                    nc.scalar.dma_start(out=fT_sl[l_out][p * 128:(p + 1) * 128, :],
                                        in_=ft[:])
                chunk_base += nch_sg

        scope = os.environ.get("KERNEL_SCOPE", "full")
        if scope == "full":
            # schedule: tab f1, AG f1, tab s1, AG s1, agg f1, tab g1, AG g1,
            # agg s1, tab f2, AG f2, agg g1, ... so each AllGather overlaps
            # the previous layer's aggregation (ping-pong table buffers).
            plans = []
            for i, (v, ln) in enumerate(LAYERS):
                nm = f"{v}{ln}"
                src = ("x", None) if ln == 1 else ("f", 2 * VIEWS.index(v))
                plans.append({"v": v, "nm": nm, "src": src, "l_out": 2 * VIEWS.index(v) + ln - 1,
                              "buf": i % 2, "ts": i % 2})

            def do_tab(i):
                p = plans[i]
                stores = list(tab_phase(p["src"][0], p["src"][1], p["nm"], p["ts"]))
                allgather_table(stores, p["ts"], p["buf"])

            def do_agg(i):
                p = plans[i]
                agg_phase(p["v"], p["nm"], p["l_out"], p["buf"])

            do_tab(0)
            do_tab(1)
            do_agg(0)
            do_tab(2)
            do_agg(1)
            do_tab(3)
            do_agg(2)
            do_tab(4)
            do_agg(3)
            do_tab(5)
            do_agg(4)
            do_agg(5)
        else:
            plans = [{"src": ("x", None), "nm": "f1", "ts": 0, "buf": 0, "v": "f",
                      "l_out": 0}]
            stores = list(tab_phase("x", None, "f1", 0))
            if scope in ("tabag", "agg1", "f1out"):
                allgather_table(stores, 0, 0)
            if scope in ("agg1", "f1out"):
                agg_phase("f", "f1", 0, 0)

        # ---- pooled -> SE attention scalars ----
        pool_red = singles.tile([128, 6], f32)
        nc.gpsimd.partition_all_reduce(pool_red[:], pooled_acc[:], 128,
                                       bass_isa.ReduceOp.add)
        nc.sync.dma_start(out=pool6_in[:], in_=pool_red[0:1, 0:6])
        nc.gpsimd.collective_compute(
            "AllReduce", mybir.AluOpType.add,
            replica_groups=[list(range(NCORES))],
            ins=[pool6_in[:]], outs=[pool6_out[:]],
        )
        pvec2 = singles.tile([6, 1], f32)
        nc.sync.dma_start(out=pvec2[:], in_=pool6_out[:])
        corr_sb = singles.tile([6, 1], f32)
        nc.sync.dma_start(out=corr_sb[:], in_=corr_in.unsqueeze(1))
        # remove pad-column relu(bias) pollution, then mean
        nc.vector.tensor_tensor(out=pvec2[:], in0=pvec2[:], in1=corr_sb[:],
                                op=mybir.AluOpType.subtract)
        nc.vector.tensor_scalar_mul(pvec2[:], pvec2[:], 1.0 / (N * FM))
        fc1wT = singles.tile([6, 30], f32)
        nc.sync.dma_start(out=fc1wT[:], in_=fc1wT_in[:])
        fc1b = singles.tile([30, 1], f32)
        nc.sync.dma_start(out=fc1b[:], in_=fc1b_in.unsqueeze(1))
        fc2wT = singles.tile([30, 6], f32)
        nc.sync.dma_start(out=fc2wT[:], in_=fc2wT_in[:])
        fc2b = singles.tile([6, 1], f32)
        nc.sync.dma_start(out=fc2b[:], in_=fc2b_in.unsqueeze(1))
        pz1 = psB.tile([30, 1], f32, tag="tabps")
        nc.tensor.matmul(pz1[:], lhsT=fc1wT[:], rhs=pvec2[:], start=True, stop=True)
        z1 = singles.tile([30, 1], f32)
        nc.vector.tensor_tensor(out=z1[:], in0=pz1[:], in1=fc1b[:], op=mybir.AluOpType.add)
        nc.vector.tensor_scalar_max(z1[:], z1[:], 0.0)
        pz2 = psB.tile([6, 1], f32, tag="tabps")
        nc.tensor.matmul(pz2[:], lhsT=fc2wT[:], rhs=z1[:], start=True, stop=True)
        z2 = singles.tile([6, 1], f32)
        nc.vector.tensor_tensor(out=z2[:], in0=pz2[:], in1=fc2b[:], op=mybir.AluOpType.add)
        av = singles.tile([6, 1], f32)
        nc.scalar.activation(out=av[:], in_=z2[:], func=mybir.ActivationFunctionType.Sigmoid)
        nc.sync.dma_start(out=a_scr[:], in_=av[:, 0])
        a_b = singles.tile([128, 6], f32)
        nc.gpsimd.dma_start(out=a_b[:], in_=a_scr[:].partition_broadcast(128))
        cnnw_b = singles.tile([128, 6], f32)
        nc.gpsimd.dma_start(out=cnnw_b[:], in_=cnnw_in.partition_broadcast(128))
        cnnb_b = singles.tile([128, 1], f32)
        nc.gpsimd.dma_start(out=cnnb_b[:], in_=cnnb_in.partition_broadcast(128))

        # ---- final combine: outT = sum_l cnnw_l * relu(a_l * fT_l) + cnn_b ----
        if scope == "f1out":
            for p in range(NPOS):
                rows = slice(p * 128, (p + 1) * 128)
                fl0 = fpo.tile([128, 128], bf16, tag="fin", name=f"fl0_{p}")
                nc.sync.dma_start(out=fl0[:], in_=fT_sl[0][rows, :])
                fo = fpo.tile([128, 128], f32, tag="ftmp", name=f"fo_{p}")
                nc.vector.tensor_copy(out=fo[:], in_=fl0[:])
                nc.sync.dma_start(out=out_d[rows, :], in_=fo[:])
        nlayers = 6 if scope == "full" else 1
        for p in range(NPOS) if scope != "f1out" else []:
            rows = slice(p * 128, (p + 1) * 128)
            acc = fpo.tile([128, 128], f32, tag="facc")
            for l in range(nlayers):
                fl = fpo.tile([128, 128], bf16, tag="fin")
                nc.sync.dma_start(out=fl[:], in_=fT_sl[l][rows, :])
                t = fpo.tile([128, 128], f32, tag="ftmp")
                nc.scalar.activation(out=t[:], in_=fl[:],
                                     func=mybir.ActivationFunctionType.Relu,
                                     scale=a_b[:, l:l + 1])
                if l == 0:
                    nc.vector.tensor_scalar_mul(acc[:], t[:], cnnw_b[:, 0:1])
                else:
                    nc.vector.tensor_scalar_mul(t[:], t[:], cnnw_b[:, l:l + 1])
                    nc.vector.tensor_tensor(out=acc[:], in0=acc[:], in1=t[:],
                                            op=mybir.AluOpType.add)
            nc.vector.tensor_scalar_add(acc[:], acc[:], cnnb_b[:, 0:1])
            nc.sync.dma_start(out=out_d[rows, :], in_=acc[:])

    nc.compile()
    if os.environ.get("KERNEL_NO_SPLIT", "0") != "1":
        _split_multiwaits(nc)
    return nc


def kernel(**inputs):
    global _last_exec_time_ns
    inputs = {k: np.asarray(v) for k, v in inputs.items()}

    meta = {}
    perview = {}
    for v in VIEWS:
        idx_arrs, dr_arrs, nm_arrs, NLO, NHI = _prep_view(
            inputs[f"edges_{v}"].astype(np.int64), inputs[f"ew_{v}"])
        meta[v] = (NLO, NHI, int((NLO + NHI).sum()))
        perview[v] = (idx_arrs, dr_arrs, nm_arrs)

    nc = _build(meta)

    xT = inputs["x_m"].T.astype(np.float32)  # [128, N]
    xT_pad = np.zeros((128, NPAD), np.float32)
    xT_pad[:, :N] = xT
    xT_pad = xT_pad.astype(bfnp)
    iota_np = np.tile(np.arange(128, dtype=np.float32), (128, 1)).astype(bfnp)
    # pad dst columns (node ids >= N, all on core 7) read relu(bias) into the
    # pooled sum; precompute the exact pollution per layer and subtract it.
    npad_cols = NPAD - N
    corr = np.array(
        [npad_cols * np.maximum(inputs[f"b_{nm}"].astype(np.float64), 0).sum()
         for nm in ["f1", "f2", "s1", "s2", "g1", "g2"]], np.float32)

    in_maps = []
    for c in range(NCORES):
        m = {
            "xT_slice": np.ascontiguousarray(xT_pad[:, c * RPC:(c + 1) * RPC]),
            "iota": iota_np,
            "fc1wT": inputs["fc1_w"].T.astype(np.float32).copy(),
            "fc1b": inputs["fc1_b"].astype(np.float32),
            "fc2wT": inputs["fc2_w"].T.astype(np.float32).copy(),
            "fc2b": inputs["fc2_b"].astype(np.float32),
            "cnnw": inputs["cnn_w"].astype(np.float32),
            "cnnb": inputs["cnn_b"].astype(np.float32),
            "corr": corr,
        }
        for nm in ["f1", "f2", "s1", "s2", "g1", "g2"]:
            m[f"W_{nm}"] = inputs[f"W_{nm}"].astype(bfnp)
            m[f"b_{nm}"] = inputs[f"b_{nm}"].astype(np.float32)
        for v in VIEWS:
            idx_arrs, dr_arrs, nm_arrs = perview[v]
            m[f"idx_{v}"] = idx_arrs[c]
            m[f"dr_{v}"] = dr_arrs[c]
            m[f"nm_{v}"] = nm_arrs[c]
        in_maps.append(m)

    trace = os.environ.get("KERNEL_TRACE", "0") == "1"
    kw = {}
    if trace:
        td = os.environ.get("KERNEL_TRACE_DIR")
        if td:
            os.makedirs(td, exist_ok=True)
            kw["tmpdir"] = td
    res = run_bass_kernel_spmd(nc, in_maps, list(range(NCORES)), trace=trace, **kw)
    _last_exec_time_ns = res.exec_time_ns
    out = np.concatenate([res.results[c]["out_slice"] for c in range(NCORES)], axis=0)
    return np.ascontiguousarray(out[:N]).astype(np.float32)


# revision 9
# speedup vs baseline: 1.1746x; 1.1746x over previous
"""GCN message-passing kernel for 8 Trainium2 NeuronCores.

Strategy: shard destination nodes across cores (6272 rows/core). Each core
aggregates all edges targeting its rows by gathering source rows from a
replicated bf16 node-feature table with SWDGE dma_gather in PREPARE_ONLY +
trigger_dma mode (descriptor generation pipelines with the DMA transfers
instead of blocking GpSimd for the full round-trip) and contracting each
128-edge chunk against a one-hot selector generated ON-CHIP by one fused DVE
tensor_scalar op per chunk: sel = (iota == dstrel) * norm, with dstrel/norm
streamed as tiny [128, C] scalar tables. The aggregation runs transposed
(psum[feat, dst]) so bias+relu+row-sum fuse into one Activation-engine op.
Layer tables are computed node-major directly (lhsT = xT tile, rhs = W; no PE
transpose) and ping-pong between two DRAM buffers so each AllGather overlaps
the previous layer's aggregation. SE attention + 1x1 conv are tiny and
replicated; the final output is produced transposed and fixed up on host.
"""
import os
import sys

sys.path.insert(0, "/opt/trn_rl_repo")

from contextlib import ExitStack

import ml_dtypes
import numpy as np

import concourse.bacc as bacc
import concourse.tile as tile
from concourse.tile import add_dep_helper
from concourse import bass_isa, mybir
from concourse.bass_utils import run_bass_kernel_spmd

N = 50000
FM = 128
E = 800000
NCORES = 8
NPOS = 49                  # 128-row tiles per core
RPC = NPOS * 128           # 6272 rows per core
NPAD = NCORES * RPC        # 50176
HALF = NPAD // 2           # 25088 (int16 gather index limit per table half)
SG = 4                     # positions per gather supergroup
VIEWS = ("f", "s", "g")
LAYERS = [("f", 1), ("s", 1), ("g", 1), ("f", 2), ("s", 2), ("g", 2)]

f32 = mybir.dt.float32
bf16 = mybir.dt.bfloat16
i16 = mybir.dt.int16
bfnp = ml_dtypes.bfloat16

_last_exec_time_ns = None


def _split_multiwaits(nc):
    """This walrus build accepts only ONE sync-wait per instruction; split
    extras into preceding same-engine single-wait NoOps (sequencer executes
    waits in program order, so semantics are preserved)."""
    n = 0
    for fn in nc.m.functions:
        for bb in fn.blocks:
            newlist = []
            for inst in bb.instructions:
                si = inst.sync_info
                if si is not None and len(si.on_wait) > 1:
                    waits = list(si.on_wait)
                    for w in waits[:-1]:
                        nop = mybir.InstNoOp(name=f"WSPL-{nc.next_id()}", ins=[], outs=[])
                        nop.engine = inst.engine
                        nop.sync_info = mybir.SyncInfo(on_wait=[w], on_update=[])
                        newlist.append(nop)
                        n += 1
                    si.on_wait = [waits[-1]]
                newlist.append(inst)
            bb.instructions = newlist
    return n


def _prep_view(edges, ew):
    """Host edge preprocessing for one view: append self-loops, compute the
    symmetric GCN normalization, shard by destination across cores, group by
    (dst tile, src half), pad runs to 128-edge chunks (uniform across cores).

    Returns (idx_arrs, dr_arrs, nm_arrs, NLO, NHI): per-core SWDGE index
    arrays plus per-edge-slot dstrel/norm scalar tables ([128, C] bf16) from
    which the device generates one-hot selector chunks on-chip."""
    src = np.concatenate([edges[0], np.arange(N, dtype=np.int64)])
    dst = np.concatenate([edges[1], np.arange(N, dtype=np.int64)])
    w = np.concatenate([ew.astype(np.float64), np.ones(N)])
    deg = np.bincount(dst, weights=w, minlength=N)
    dis = 1.0 / np.sqrt(deg)
    norm = (dis[src] * w * dis[dst]).astype(np.float32)

    core = dst // RPC
    pos = (dst % RPC) // 128
    dstrel = (dst % 128).astype(np.int64)
    half = (src >= HALF).astype(np.int64)
    idx = (src - HALF * half).astype(np.int16)

    # counts[c, p, h]
    key = (core * NPOS + pos) * 2 + half
    counts = np.bincount(key, minlength=NCORES * NPOS * 2).reshape(NCORES, NPOS, 2)
    chunks = -(-counts // 128)  # ceil
    NLO = chunks[:, :, 0].max(axis=0)
    NHI = chunks[:, :, 1].max(axis=0)

    order = np.lexsort((half, pos, core))
    norm_s, dstrel_s, idx_s, key_s = (
        norm[order], dstrel[order], idx[order], key[order])
    starts = np.searchsorted(key_s, np.arange(NCORES * NPOS * 2))
    ends = np.searchsorted(key_s, np.arange(NCORES * NPOS * 2), side="right")

    C = int((NLO + NHI).sum())
    idx_arrs, dr_arrs, nm_arrs = [], [], []
    sgs = [list(range(s, min(s + SG, NPOS))) for s in range(0, NPOS, SG)]
    for c in range(NCORES):
        idx_a = np.zeros(C * 128, np.int16)
        dr_a = np.zeros(C * 128, np.int64)
        w_a = np.zeros(C * 128, np.float32)
        off = 0
        for sg in sgs:
            for h in range(2):
                for p in sg:
                    k = (c * NPOS + p) * 2 + h
                    s0, e0 = starts[k], ends[k]
                    n = e0 - s0
                    nch = (NLO if h == 0 else NHI)[p]
                    idx_a[off:off + n] = idx_s[s0:e0]
                    dr_a[off:off + n] = dstrel_s[s0:e0]
                    w_a[off:off + n] = norm_s[s0:e0]
                    off += nch * 128
        assert off == C * 128
        # device layouts
        idx_wrapped = np.tile(idx_a.reshape(-1, 16).T, (8, 1)).copy()  # [128, C*8]
        dr_dev = np.ascontiguousarray(dr_a.reshape(C, 128).T).astype(bfnp)
        nm_dev = np.ascontiguousarray(w_a.reshape(C, 128).T).astype(bfnp)
        idx_arrs.append(idx_wrapped)
        dr_arrs.append(dr_dev)
        nm_arrs.append(nm_dev)
    return idx_arrs, dr_arrs, nm_arrs, NLO.astype(int), NHI.astype(int)


def _build(meta):
    """Build the SPMD program. meta[v] = (NLO, NHI, C) per view."""
    nc = bacc.Bacc("TRN2", target_bir_lowering=False, debug=False,
                   num_devices=NCORES,
                   dynamic_dma_scratch_size=int(os.environ.get("KERNEL_DMA_SCRATCH", "16384")))

    # ---- I/O ----
    xT_in = nc.dram_tensor("xT_slice", [128, RPC], bf16, kind="ExternalInput").ap()
    W_in, b_in = {}, {}
    for nm in ["f1", "f2", "s1", "s2", "g1", "g2"]:
        W_in[nm] = nc.dram_tensor(f"W_{nm}", [FM, FM], bf16, kind="ExternalInput").ap()
        b_in[nm] = nc.dram_tensor(f"b_{nm}", [FM], f32, kind="ExternalInput").ap()
    idx_in, dr_in, nm_in = {}, {}, {}
    for v in VIEWS:
        C = meta[v][2]
        idx_in[v] = nc.dram_tensor(f"idx_{v}", [128, C * 8], i16, kind="ExternalInput").ap()
        dr_in[v] = nc.dram_tensor(f"dr_{v}", [128, C], bf16, kind="ExternalInput").ap()
        nm_in[v] = nc.dram_tensor(f"nm_{v}", [128, C], bf16, kind="ExternalInput").ap()
    iota_in = nc.dram_tensor("iota", [128, 128], bf16, kind="ExternalInput").ap()
    fc1wT_in = nc.dram_tensor("fc1wT", [6, 30], f32, kind="ExternalInput").ap()
    fc1b_in = nc.dram_tensor("fc1b", [30], f32, kind="ExternalInput").ap()
    fc2wT_in = nc.dram_tensor("fc2wT", [30, 6], f32, kind="ExternalInput").ap()
    fc2b_in = nc.dram_tensor("fc2b", [6], f32, kind="ExternalInput").ap()
    cnnw_in = nc.dram_tensor("cnnw", [6], f32, kind="ExternalInput").ap()
    cnnb_in = nc.dram_tensor("cnnb", [1], f32, kind="ExternalInput").ap()
    corr_in = nc.dram_tensor("corr", [6], f32, kind="ExternalInput").ap()
    out_d = nc.dram_tensor("out_slice", [RPC, FM], f32, kind="ExternalOutput").ap()

    dma_sem = nc.alloc_semaphore("gather_dma")
    NGSEM = 4
    gsems = [nc.alloc_semaphore(f"gsem{i}") for i in range(NGSEM)]

    with tile.TileContext(nc) as tc, ExitStack() as ctx:
        singles = ctx.enter_context(tc.tile_pool(name="singles", bufs=1))
        pool = ctx.enter_context(tc.tile_pool(name="pool", bufs=3))
        selp = ctx.enter_context(tc.tile_pool(name="selp", bufs=2))
        gpo = ctx.enter_context(tc.tile_pool(name="gpo", bufs=2))
        fpo = ctx.enter_context(tc.tile_pool(name="fpo", bufs=4))
        psA = ctx.enter_context(tc.tile_pool(name="psA", bufs=4, space="PSUM"))
        psB = ctx.enter_context(tc.tile_pool(name="psB", bufs=2, space="PSUM"))
        dram = ctx.enter_context(tc.tile_pool(name="dram", bufs=1, space="DRAM"))

        tables = [dram.tile([NPAD, FM], bf16, name=f"table{i}") for i in range(2)]
        tab_slices = [dram.tile([RPC, FM], bf16, name=f"tab_slice{i}") for i in range(2)]
        fT_sl = [dram.tile([RPC, 128], bf16, name=f"fT_sl{i}") for i in range(6)]
        pool6_in = dram.tile([6, 1], f32, name="pool6_in")
        pool6_out = dram.tile([6, 1], f32, name="pool6_out")
        a_scr = dram.tile([6], f32, name="a_scr")

        # ---- constants ----
        iota_sb = singles.tile([128, 128], bf16)
        nc.sync.dma_start(out=iota_sb[:], in_=iota_in[:])
        W_sb, bb_bc = {}, {}
        for nm in ["f1", "f2", "s1", "s2", "g1", "g2"]:
            W_sb[nm] = singles.tile([FM, FM], bf16, tag=f"W_{nm}", name=f"Wsb_{nm}")
            nc.sync.dma_start(out=W_sb[nm][:], in_=W_in[nm][:])
            bb_bc[nm] = singles.tile([128, FM], f32, tag=f"bb_{nm}", name=f"bbsb_{nm}")
            nc.gpsimd.dma_start(out=bb_bc[nm][:], in_=b_in[nm].partition_broadcast(128))
        pooled_acc = singles.tile([128, 6], f32)
        nc.vector.memset(pooled_acc[:], 0.0)

        idx_sb, dr_sb, nm_sb = {}, {}, {}
        for v in VIEWS:
            C = meta[v][2]
            idx_sb[v] = singles.tile([128, C * 8], i16, tag=f"idx_{v}", name=f"idxsb_{v}")
            nc.sync.dma_start(out=idx_sb[v][:], in_=idx_in[v][:])
            dr_sb[v] = singles.tile([128, C], bf16, tag=f"dr_{v}", name=f"drsb_{v}")
            nc.sync.dma_start(out=dr_sb[v][:], in_=dr_in[v][:])
            nm_sb[v] = singles.tile([128, C], bf16, tag=f"nm_{v}", name=f"nmsb_{v}")
            nc.sync.dma_start(out=nm_sb[v][:], in_=nm_in[v][:])

        # per-table-buffer state for manual collective/gather dep tracking
        # (custom-DMA APs over DRAM pool tiles are not reliably dep-tracked)
        tabst = [{"ag": None, "preps": []} for _ in range(2)]
        slice_ag = [None, None]   # last AllGather reading tab_slices[i]
        gstate = {"sg": 0, "cum": [0] * NGSEM}

        def tab_phase(src_kind, vsrc_l, Wn, ts):
            """tab_slices[ts] = cast_bf16(src @ W) for own rows, node-major.
            src 'x': xT input; src 'f': fT_sl[vsrc_l] (both [feat, node])."""
            war = slice_ag[ts]
            for p in range(NPOS):
                cols = slice(p * 128, (p + 1) * 128)
                t_fn = pool.tile([128, 128], bf16, tag="tabin")
                if src_kind == "x":
                    ld = nc.sync.dma_start(out=t_fn[:], in_=xT_in[:, cols])
                else:
                    ld = nc.sync.dma_start_transpose(
                        out=t_fn[:], in_=fT_sl[vsrc_l][p * 128:(p + 1) * 128, :])
                pm = psB.tile([128, 128], f32, tag="tabps")
                nc.tensor.matmul(pm[:], lhsT=t_fn[:], rhs=W_sb[Wn][:], start=True, stop=True)
                tb = pool.tile([128, 128], bf16, tag="tbf")
                nc.scalar.copy(out=tb[:], in_=pm[:])
                st = nc.sync.dma_start(out=tab_slices[ts][p * 128:(p + 1) * 128, :], in_=tb[:])
                if war is not None:
                    add_dep_helper(st.ins, war.ins, reason="tab_slice WAR")
                yield st

        def allgather_table(tab_stores, ts, buf):
            ag = nc.gpsimd.collective_compute(
                "AllGather", mybir.AluOpType.bypass,
                replica_groups=[list(range(NCORES))],
                ins=[tab_slices[ts][:]], outs=[tables[buf][:]],
            )
            for st in tab_stores:
                add_dep_helper(ag.ins, st.ins, reason="tab_slice RAW")
            for g in tabst[buf]["preps"]:
                add_dep_helper(ag.ins, g.ins, reason="table WAR")
            # WAR vs in-flight prepared gathers: prep engine-ticks only cover
            # descriptor GENERATION; wait for the transfers via the gather
            # sems, attached directly to the collective so the scheduler
            # cannot float the wait away from it.
            for i in range(NGSEM):
                if gstate["cum"][i] > 0:
                    ag.wait_op(gsems[i], gstate["cum"][i], "sem-ge", check=False)
            tabst[buf] = {"ag": ag, "preps": []}
            slice_ag[ts] = ag

        def agg_phase(v, Wn, l_out, buf):
            NLO, NHI, C = meta[v]
            ag = tabst[buf]["ag"]
            sgs = [list(range(s, min(s + SG, NPOS))) for s in range(0, NPOS, SG)]
            chunk_base = 0  # global chunk counter
            idx_col = 0     # column offset into idx_sb (units of 16 idxs)
            for sg in sgs:
                nlo = int(sum(NLO[p] for p in sg))
                nhi = int(sum(NHI[p] for p in sg))
                nch_sg = nlo + nhi
                # generate this supergroup's selector chunks on-chip in two
                # batched DVE ops over broadcast APs:
                # sel[e, c, d] = (iota[e, d] == dr[e, c]) * nm[e, c]
                nch_c = max(nch_sg, 1)
                selsb = selp.tile([128, nch_c, 128], bf16, tag="sel")
                iota3 = iota_sb[:].unsqueeze(1).broadcast_to([128, nch_c, 128])
                dr3 = dr_sb[v][:, chunk_base:chunk_base + nch_c].unsqueeze(2) \
                    .broadcast_to([128, nch_c, 128])
                nm3 = nm_sb[v][:, chunk_base:chunk_base + nch_c].unsqueeze(2) \
                    .broadcast_to([128, nch_c, 128])
                nc.vector.tensor_tensor(out=selsb[:], in0=iota3, in1=dr3,
                                        op=mybir.AluOpType.is_equal)
                nc.vector.tensor_tensor(out=selsb[:], in0=selsb[:], in1=nm3,
                                        op=mybir.AluOpType.mult)
                glo = gpo.tile([128, max(nlo, 1), 128], bf16, tag="glo")
                ghi = gpo.tile([128, max(nhi, 1), 128], bf16, tag="ghi")
                sync_gather = os.environ.get("KERNEL_SYNC_GATHER", "0") == "1"
                GMAXC = int(os.environ.get("KERNEL_GMAXC", "8"))
                trig_batch = os.environ.get("KERNEL_TRIG_BATCH", "prep")
                si = gstate["sg"] % NGSEM
                gstate["sg"] += 1
                nprep_sg = 0
                for half_i, (nh, gt, lohi) in enumerate(
                        (((nlo, glo, (0, HALF)), (nhi, ghi, (HALF, NPAD))))):
                    for g0 in range(0, nh, GMAXC):
                        gn = min(GMAXC, nh - g0)
                        if sync_gather:
                            gi = nc.gpsimd.dma_gather(
                                out_ap=gt[:, g0:g0 + gn, :],
                                in_ap=tables[buf][lohi[0]:lohi[1], :],
                                idxs_ap=idx_sb[v][:, idx_col:idx_col + gn * 8],
                                num_idxs=gn * 128, num_idxs_reg=gn * 128, elem_size=128,
                            )
                            add_dep_helper(gi.ins, ag.ins, reason="table RAW")
                            tabst[buf]["preps"].append(gi)
                        else:
                            gi = nc.gpsimd.dma_gather(
                                out_ap=gt[:, g0:g0 + gn, :],
                                in_ap=tables[buf][lohi[0]:lohi[1], :],
                                idxs_ap=idx_sb[v][:, idx_col:idx_col + gn * 8],
                                num_idxs=gn * 128, num_idxs_reg=gn * 128, elem_size=128,
                                prepare_only=True, sem=gsems[si],
                            )
                            add_dep_helper(gi.ins, ag.ins, reason="table RAW")
                            if trig_batch == "prep":
                                tr = nc.gpsimd.trigger_dma(count=None)
                                add_dep_helper(tr.ins, ag.ins, reason="table RAW @trigger")
                            nprep_sg += 1
                        idx_col += gn * 8
                if not sync_gather and nprep_sg and trig_batch == "sg":
                    tr = nc.gpsimd.trigger_dma(count=None)
                    add_dep_helper(tr.ins, ag.ins, reason="table RAW @trigger")
                if not sync_gather and nprep_sg:
                    # Tile's gen_mode==1 DMASW lane protocol releases consumers
                    # one prep early; gate this supergroup's matmuls on the
                    # explicit per-supergroup DMA-completion sem instead (the
                    # wait is attached to the first matmul of each position).
                    gstate["cum"][si] += 16 * nprep_sg
                # chunk order in sel array: [lo(p0)..lo(pk)] then [hi(p0)..hi(pk)]
                lo_off, off = {}, 0
                for p in sg:
                    lo_off[p] = off
                    off += int(NLO[p])
                hi_off, off = {}, 0
                for p in sg:
                    hi_off[p] = off
                    off += int(NHI[p])
                for p in sg:
                    # psum[dst, feat]: lhsT = one-hot selector (stationary; its
                    # DVE producer deps land on the Ldweights), rhs = gathered
                    # rows (moving; rhs deps land on the matmul itself, where
                    # the explicit gather-completion wait is attached).
                    ps = psA.tile([128, 128], f32, tag="agg")
                    nch = int(NLO[p] + NHI[p])
                    ci = 0
                    for k in range(int(NLO[p])):
                        cg = lo_off[p] + k  # sel col block within supergroup
                        mm = nc.tensor.matmul(ps[:], rhs=glo[:, lo_off[p] + k, :],
                                              lhsT=selsb[:, cg, :],
                                              start=(ci == 0), stop=(ci == nch - 1))
                        if ci == 0 and not sync_gather and gstate["cum"][si] > 0:
                            mm.wait_op(gsems[si], gstate["cum"][si], "sem-ge", check=False)
                        ci += 1
                    for k in range(int(NHI[p])):
                        cg = nlo + hi_off[p] + k
                        mm = nc.tensor.matmul(ps[:], rhs=ghi[:, hi_off[p] + k, :],
                                              lhsT=selsb[:, cg, :],
                                              start=(ci == 0), stop=(ci == nch - 1))
                        if ci == 0 and not sync_gather and gstate["cum"][si] > 0:
                            mm.wait_op(gsems[si], gstate["cum"][si], "sem-ge", check=False)
                        ci += 1
                    # postprocess: fT = relu(agg + b), fused row-sum for pooling
                    tadd = fpo.tile([128, 128], f32, tag="tadd")
                    nc.vector.tensor_tensor(out=tadd[:], in0=ps[:], in1=bb_bc[Wn][:],
                                            op=mybir.AluOpType.add)
                    ft = fpo.tile([128, 128], bf16, tag="ftile")
                    racc = fpo.tile([128, 1], f32, tag="racc")
                    nc.scalar.activation(
                        out=ft[:], in_=tadd[:], func=mybir.ActivationFunctionType.Relu,
                        accum_out=racc[:])
                    nc.vector.tensor_tensor(out=pooled_acc[:, l_out:l_out + 1],
                                            in0=pooled_acc[:, l_out:l_out + 1],
                                            in1=racc[:], op=mybir.AluOpType.add)
                    nc.scalar.dma_start(out=fT_sl[l_out][p * 128:(p + 1) * 128, :],
                                        in_=ft[:])
                chunk_base += nch_sg

        scope = os.environ.get("KERNEL_SCOPE", "full")
        if scope == "full":
            # schedule: tab f1, AG f1, tab s1, AG s1, agg f1, tab g1, AG g1,
            # agg s1, tab f2, AG f2, agg g1, ... so each AllGather overlaps
            # the previous layer's aggregation (ping-pong table buffers).
            plans = []
            for i, (v, ln) in enumerate(LAYERS):
                nm = f"{v}{ln}"
                src = ("x", None) if ln == 1 else ("f", 2 * VIEWS.index(v))
                plans.append({"v": v, "nm": nm, "src": src, "l_out": 2 * VIEWS.index(v) + ln - 1,
                              "buf": i % 2, "ts": i % 2})

            def do_tab(i):
                p = plans[i]
                stores = list(tab_phase(p["src"][0], p["src"][1], p["nm"], p["ts"]))
                allgather_table(stores, p["ts"], p["buf"])

            def do_agg(i):
                p = plans[i]
                agg_phase(p["v"], p["nm"], p["l_out"], p["buf"])

            do_tab(0)
            do_tab(1)
            do_agg(0)
            do_tab(2)
            do_agg(1)
            do_tab(3)
            do_agg(2)
            do_tab(4)
            do_agg(3)
            do_tab(5)
            do_agg(4)
            do_agg(5)
        else:
            plans = [{"src": ("x", None), "nm": "f1", "ts": 0, "buf": 0, "v": "f",
                      "l_out": 0}]
            stores = list(tab_phase("x", None, "f1", 0))
            if scope in ("tabag", "agg1", "f1out"):
                allgather_table(stores, 0, 0)
            if scope in ("agg1", "f1out"):
                agg_phase("f", "f1", 0, 0)

        # ---- pooled -> SE attention scalars ----
        pool_red = singles.tile([128, 6], f32)
        nc.gpsimd.partition_all_reduce(pool_red[:], pooled_acc[:], 128,
                                       bass_isa.ReduceOp.add)
        nc.sync.dma_start(out=pool6_in[:], in_=pool_red[0:1, 0:6])
        nc.gpsimd.collective_compute(
            "AllReduce", mybir.AluOpType.add,
            replica_groups=[list(range(NCORES))],
            ins=[pool6_in[:]], outs=[pool6_out[:]],
        )
        pvec2 = singles.tile([6, 1], f32)
        nc.sync.dma_start(out=pvec2[:], in_=pool6_out[:])
        corr_sb = singles.tile([6, 1], f32)
        nc.sync.dma_start(out=corr_sb[:], in_=corr_in.unsqueeze(1))
        # remove pad-column relu(bias) pollution, then mean
        nc.vector.tensor_tensor(out=pvec2[:], in0=pvec2[:], in1=corr_sb[:],
                                op=mybir.AluOpType.subtract)
        nc.vector.tensor_scalar_mul(pvec2[:], pvec2[:], 1.0 / (N * FM))
        fc1wT = singles.tile([6, 30], f32)
        nc.sync.dma_start(out=fc1wT[:], in_=fc1wT_in[:])
        fc1b = singles.tile([30, 1], f32)
        nc.sync.dma_start(out=fc1b[:], in_=fc1b_in.unsqueeze(1))
        fc2wT = singles.tile([30, 6], f32)
        nc.sync.dma_start(out=fc2wT[:], in_=fc2wT_in[:])
        fc2b = singles.tile([6, 1], f32)
        nc.sync.dma_start(out=fc2b[:], in_=fc2b_in.unsqueeze(1))
        pz1 = psB.tile([30, 1], f32, tag="tabps")
        nc.tensor.matmul(pz1[:], lhsT=fc1wT[:], rhs=pvec2[:], start=True, stop=True)
        z1 = singles.tile([30, 1], f32)
        nc.vector.tensor_tensor(out=z1[:], in0=pz1[:], in1=fc1b[:], op=mybir.AluOpType.add)
        nc.vector.tensor_scalar_max(z1[:], z1[:], 0.0)
        pz2 = psB.tile([6, 1], f32, tag="tabps")
        nc.tensor.matmul(pz2[:], lhsT=fc2wT[:], rhs=z1[:], start=True, stop=True)
        z2 = singles.tile([6, 1], f32)
        nc.vector.tensor_tensor(out=z2[:], in0=pz2[:], in1=fc2b[:], op=mybir.AluOpType.add)
        av = singles.tile([6, 1], f32)
        nc.scalar.activation(out=av[:], in_=z2[:], func=mybir.ActivationFunctionType.Sigmoid)
        nc.sync.dma_start(out=a_scr[:], in_=av[:, 0])
        a_b = singles.tile([128, 6], f32)
        nc.gpsimd.dma_start(out=a_b[:], in_=a_scr[:].partition_broadcast(128))
        cnnw_b = singles.tile([128, 6], f32)
        nc.gpsimd.dma_start(out=cnnw_b[:], in_=cnnw_in.partition_broadcast(128))
        cnnb_b = singles.tile([128, 1], f32)
        nc.gpsimd.dma_start(out=cnnb_b[:], in_=cnnb_in.partition_broadcast(128))

        # ---- final combine: outT = sum_l cnnw_l * relu(a_l * fT_l) + cnn_b ----
        if scope == "f1out":
            for p in range(NPOS):
                rows = slice(p * 128, (p + 1) * 128)
                fl0 = fpo.tile([128, 128], bf16, tag="fin", name=f"fl0_{p}")
                nc.sync.dma_start(out=fl0[:], in_=fT_sl[0][rows, :])
                fo = fpo.tile([128, 128], f32, tag="ftmp", name=f"fo_{p}")
                nc.vector.tensor_copy(out=fo[:], in_=fl0[:])
                nc.sync.dma_start(out=out_d[rows, :], in_=fo[:])
        nlayers = 6 if scope == "full" else 1
        for p in range(NPOS) if scope != "f1out" else []:
            rows = slice(p * 128, (p + 1) * 128)
            acc = fpo.tile([128, 128], f32, tag="facc")
            for l in range(nlayers):
                fl = fpo.tile([128, 128], bf16, tag="fin")
                nc.sync.dma_start(out=fl[:], in_=fT_sl[l][rows, :])
                t = fpo.tile([128, 128], f32, tag="ftmp")
                nc.scalar.activation(out=t[:], in_=fl[:],
                                     func=mybir.ActivationFunctionType.Relu,
                                     scale=a_b[:, l:l + 1])
                if l == 0:
                    nc.vector.tensor_scalar_mul(acc[:], t[:], cnnw_b[:, 0:1])
                else:
                    nc.vector.tensor_scalar_mul(t[:], t[:], cnnw_b[:, l:l + 1])
                    nc.vector.tensor_tensor(out=acc[:], in0=acc[:], in1=t[:],
                                            op=mybir.AluOpType.add)
            nc.vector.tensor_scalar_add(acc[:], acc[:], cnnb_b[:, 0:1])
            nc.sync.dma_start(out=out_d[rows, :], in_=acc[:])

    nc.compile()
    if os.environ.get("KERNEL_NO_SPLIT", "0") != "1":
        _split_multiwaits(nc)
    return nc


def kernel(**inputs):
    global _last_exec_time_ns
    inputs = {k: np.asarray(v) for k, v in inputs.items()}

    meta = {}
    perview = {}
    for v in VIEWS:
        idx_arrs, dr_arrs, nm_arrs, NLO, NHI = _prep_view(
            inputs[f"edges_{v}"].astype(np.int64), inputs[f"ew_{v}"])
        meta[v] = (NLO, NHI, int((NLO + NHI).sum()))
        perview[v] = (idx_arrs, dr_arrs, nm_arrs)

    nc = _build(meta)

    xT = inputs["x_m"].T.astype(np.float32)  # [128, N]
    xT_pad = np.zeros((128, NPAD), np.float32)
    xT_pad[:, :N] = xT
    xT_pad = xT_pad.astype(bfnp)
    iota_np = np.tile(np.arange(128, dtype=np.float32), (128, 1)).astype(bfnp)
    # pad dst columns (node ids >= N, all on core 7) read relu(bias) into the
    # pooled sum; precompute the exact pollution per layer and subtract it.
    npad_cols = NPAD - N
    corr = np.array(
        [npad_cols * np.maximum(inputs[f"b_{nm}"].astype(np.float64), 0).sum()
         for nm in ["f1", "f2", "s1", "s2", "g1", "g2"]], np.float32)

    in_maps = []
    for c in range(NCORES):
        m = {
            "xT_slice": np.ascontiguousarray(xT_pad[:, c * RPC:(c + 1) * RPC]),
            "iota": iota_np,
            "fc1wT": inputs["fc1_w"].T.astype(np.float32).copy(),
            "fc1b": inputs["fc1_b"].astype(np.float32),
            "fc2wT": inputs["fc2_w"].T.astype(np.float32).copy(),
            "fc2b": inputs["fc2_b"].astype(np.float32),
            "cnnw": inputs["cnn_w"].astype(np.float32),
            "cnnb": inputs["cnn_b"].astype(np.float32),
            "corr": corr,
        }
        for nm in ["f1", "f2", "s1", "s2", "g1", "g2"]:
            m[f"W_{nm}"] = inputs[f"W_{nm}"].astype(bfnp)
            m[f"b_{nm}"] = inputs[f"b_{nm}"].astype(np.float32)
        for v in VIEWS:
            idx_arrs, dr_arrs, nm_arrs = perview[v]
            m[f"idx_{v}"] = idx_arrs[c]
            m[f"dr_{v}"] = dr_arrs[c]
            m[f"nm_{v}"] = nm_arrs[c]
        in_maps.append(m)

    trace = os.environ.get("KERNEL_TRACE", "0") == "1"
    kw = {}
    if trace:
        td = os.environ.get("KERNEL_TRACE_DIR")
        if td:
            os.makedirs(td, exist_ok=True)
            kw["tmpdir"] = td
    res = run_bass_kernel_spmd(nc, in_maps, list(range(NCORES)), trace=trace, **kw)
    _last_exec_time_ns = res.exec_time_ns
    out = np.concatenate([res.results[c]["out_slice"] for c in range(NCORES)], axis=0)
    return np.ascontiguousarray(out[:N]).astype(np.float32)


# revision 10
# speedup vs baseline: 1.3786x; 1.1737x over previous
"""GCN message-passing kernel for 8 Trainium2 NeuronCores.

Strategy: shard destination nodes across cores (6272 rows/core). Each core
aggregates all edges targeting its rows by gathering source rows from a
replicated bf16 node-feature table (SWDGE dma_gather, prepare_only +
trigger_dma so the Pool engine pipelines descriptor-gen with transfers) and
contracting each 128-edge chunk against a host-precomputed one-hot selector
(streamed from HBM) on the PE array. The aggregation runs transposed
(psum[feat, dst]) so bias+relu+row-sum fuse into one Activation-engine op.
Layer tables ping-pong between two DRAM buffers so each AllGather overlaps
the previous layer's aggregation. SE attention + 1x1 conv are tiny and
replicated; the final output is produced transposed and fixed up on host.
"""
import os
import sys

sys.path.insert(0, "/opt/trn_rl_repo")

from contextlib import ExitStack

import ml_dtypes
import numpy as np

import concourse.bacc as bacc
import concourse.tile as tile
from concourse.tile import add_dep_helper
from concourse import bass_isa, mybir
from concourse.bass_utils import run_bass_kernel_spmd

N = 50000
FM = 128
E = 800000
NCORES = 8
NPOS = 49                  # 128-row tiles per core
RPC = NPOS * 128           # 6272 rows per core
NPAD = NCORES * RPC        # 50176
HALF = NPAD // 2           # 25088 (int16 gather index limit per table half)
SG = 4                     # positions per gather supergroup
VIEWS = ("f", "s", "g")
LAYERS = [("f", 1), ("s", 1), ("g", 1), ("f", 2), ("s", 2), ("g", 2)]

f32 = mybir.dt.float32
bf16 = mybir.dt.bfloat16
i16 = mybir.dt.int16
bfnp = ml_dtypes.bfloat16

_last_exec_time_ns = None


def _split_multiwaits(nc):
    """This walrus build accepts only ONE sync-wait per instruction; split
    extras into preceding same-engine single-wait NoOps (sequencer executes
    waits in program order, so semantics are preserved)."""
    n = 0
    for fn in nc.m.functions:
        for bb in fn.blocks:
            newlist = []
            for inst in bb.instructions:
                si = inst.sync_info
                if si is not None and len(si.on_wait) > 1:
                    waits = list(si.on_wait)
                    for w in waits[:-1]:
                        nop = mybir.InstNoOp(name=f"WSPL-{nc.next_id()}", ins=[], outs=[])
                        nop.engine = inst.engine
                        nop.sync_info = mybir.SyncInfo(on_wait=[w], on_update=[])
                        newlist.append(nop)
                        n += 1
                    si.on_wait = [waits[-1]]
                newlist.append(inst)
            bb.instructions = newlist
    return n


def _prep_view(edges, ew):
    """Host edge preprocessing for one view: append self-loops, compute the
    symmetric GCN normalization, shard by destination across cores, group by
    (dst tile, src half), pad runs to 128-edge chunks (uniform across cores).

    Returns (idx_arrs, sel_arrs, NLO, NHI): per-core SWDGE index arrays and
    precomputed one-hot selector chunks ([128 edge-slot partitions, C*128
    dst columns], bf16, selector value = the edge's GCN norm weight)."""
    src = np.concatenate([edges[0], np.arange(N, dtype=np.int64)])
    dst = np.concatenate([edges[1], np.arange(N, dtype=np.int64)])
    w = np.concatenate([ew.astype(np.float64), np.ones(N)])
    deg = np.bincount(dst, weights=w, minlength=N)
    dis = 1.0 / np.sqrt(deg)
    norm = (dis[src] * w * dis[dst]).astype(np.float32)

    core = dst // RPC
    pos = (dst % RPC) // 128
    dstrel = (dst % 128).astype(np.int64)
    half = (src >= HALF).astype(np.int64)
    idx = (src - HALF * half).astype(np.int16)

    # counts[c, p, h]
    key = (core * NPOS + pos) * 2 + half
    counts = np.bincount(key, minlength=NCORES * NPOS * 2).reshape(NCORES, NPOS, 2)
    chunks = -(-counts // 128)  # ceil
    NLO = chunks[:, :, 0].max(axis=0)
    NHI = chunks[:, :, 1].max(axis=0)

    order = np.lexsort((half, pos, core))
    norm_s, dstrel_s, idx_s, key_s = (
        norm[order], dstrel[order], idx[order], key[order])
    starts = np.searchsorted(key_s, np.arange(NCORES * NPOS * 2))
    ends = np.searchsorted(key_s, np.arange(NCORES * NPOS * 2), side="right")

    C = int((NLO + NHI).sum())
    idx_arrs, sel_arrs = [], []
    sgs = [list(range(s, min(s + SG, NPOS))) for s in range(0, NPOS, SG)]
    for c in range(NCORES):
        idx_a = np.zeros(C * 128, np.int16)
        dr_a = np.zeros(C * 128, np.int64)
        w_a = np.zeros(C * 128, np.float32)
        off = 0
        for sg in sgs:
            for h in range(2):
                for p in sg:
                    k = (c * NPOS + p) * 2 + h
                    s0, e0 = starts[k], ends[k]
                    n = e0 - s0
                    nch = (NLO if h == 0 else NHI)[p]
                    idx_a[off:off + n] = idx_s[s0:e0]
                    dr_a[off:off + n] = dstrel_s[s0:e0]
                    w_a[off:off + n] = norm_s[s0:e0]
                    off += nch * 128
        assert off == C * 128
        # device layouts
        idx_wrapped = np.tile(idx_a.reshape(-1, 16).T, (8, 1)).copy()  # [128, C*8]
        sel_flat = np.zeros((C * 128, 128), np.float32)
        sel_flat[np.arange(C * 128), dr_a] = w_a
        sel_dev = np.ascontiguousarray(
            sel_flat.reshape(C, 128, 128).transpose(1, 0, 2).reshape(128, C * 128)
        ).astype(bfnp)
        idx_arrs.append(idx_wrapped)
        sel_arrs.append(sel_dev)
    return idx_arrs, sel_arrs, NLO.astype(int), NHI.astype(int)


def _build(meta):
    """Build the SPMD program. meta[v] = (NLO, NHI, C) per view."""
    nc = bacc.Bacc("TRN2", target_bir_lowering=False, debug=False,
                   num_devices=NCORES)

    # ---- I/O ----
    xT_in = nc.dram_tensor("xT_slice", [128, RPC], bf16, kind="ExternalInput").ap()
    W_in, b_in = {}, {}
    for nm in ["f1", "f2", "s1", "s2", "g1", "g2"]:
        W_in[nm] = nc.dram_tensor(f"W_{nm}", [FM, FM], bf16, kind="ExternalInput").ap()
        b_in[nm] = nc.dram_tensor(f"b_{nm}", [FM], f32, kind="ExternalInput").ap()
    idx_in, sel_in = {}, {}
    for v in VIEWS:
        C = meta[v][2]
        idx_in[v] = nc.dram_tensor(f"idx_{v}", [128, C * 8], i16, kind="ExternalInput").ap()
        sel_in[v] = nc.dram_tensor(f"sel_{v}", [128, C * 128], bf16, kind="ExternalInput").ap()
    ident_in = nc.dram_tensor("ident", [128, 128], bf16, kind="ExternalInput").ap()
    fc1wT_in = nc.dram_tensor("fc1wT", [6, 30], f32, kind="ExternalInput").ap()
    fc1b_in = nc.dram_tensor("fc1b", [30], f32, kind="ExternalInput").ap()
    fc2wT_in = nc.dram_tensor("fc2wT", [30, 6], f32, kind="ExternalInput").ap()
    fc2b_in = nc.dram_tensor("fc2b", [6], f32, kind="ExternalInput").ap()
    cnnw_in = nc.dram_tensor("cnnw", [6], f32, kind="ExternalInput").ap()
    cnnb_in = nc.dram_tensor("cnnb", [1], f32, kind="ExternalInput").ap()
    corr_in = nc.dram_tensor("corr", [6], f32, kind="ExternalInput").ap()
    out_d = nc.dram_tensor("out_slice", [FM, RPC], f32, kind="ExternalOutput").ap()

    dma_sem = nc.alloc_semaphore("gather_dma")

    with tile.TileContext(nc) as tc, ExitStack() as ctx:
        singles = ctx.enter_context(tc.tile_pool(name="singles", bufs=1))
        pool = ctx.enter_context(tc.tile_pool(name="pool", bufs=3))
        selp = ctx.enter_context(tc.tile_pool(name="selp", bufs=2))
        gpo = ctx.enter_context(tc.tile_pool(name="gpo", bufs=2))
        fpo = ctx.enter_context(tc.tile_pool(name="fpo", bufs=4))
        psA = ctx.enter_context(tc.tile_pool(name="psA", bufs=4, space="PSUM"))
        psB = ctx.enter_context(tc.tile_pool(name="psB", bufs=2, space="PSUM"))
        dram = ctx.enter_context(tc.tile_pool(name="dram", bufs=1, space="DRAM"))

        tables = [dram.tile([NPAD, FM], bf16, name=f"table{i}") for i in range(2)]
        tab_slices = [dram.tile([RPC, FM], bf16, name=f"tab_slice{i}") for i in range(2)]
        fT_sl = [dram.tile([128, RPC], bf16, name=f"fT_sl{i}") for i in range(6)]
        pool6_in = dram.tile([6, 1], f32, name="pool6_in")
        pool6_out = dram.tile([6, 1], f32, name="pool6_out")
        a_scr = dram.tile([6], f32, name="a_scr")

        # ---- constants ----
        ident = singles.tile([128, 128], bf16)
        nc.sync.dma_start(out=ident[:], in_=ident_in[:])
        identf = singles.tile([128, 128], f32)
        nc.vector.tensor_copy(out=identf[:], in_=ident[:])
        W_sb, bb_sb = {}, {}
        for nm in ["f1", "f2", "s1", "s2", "g1", "g2"]:
            W_sb[nm] = singles.tile([FM, FM], bf16, tag=f"W_{nm}", name=f"Wsb_{nm}")
            nc.sync.dma_start(out=W_sb[nm][:], in_=W_in[nm][:])
            bb_sb[nm] = singles.tile([FM, 1], f32, tag=f"bb_{nm}", name=f"bbsb_{nm}")
            nc.sync.dma_start(out=bb_sb[nm][:], in_=b_in[nm].unsqueeze(1))
        pooled_acc = singles.tile([128, 6], f32)
        nc.vector.memset(pooled_acc[:], 0.0)

        idx_sb = {}
        for v in VIEWS:
            C = meta[v][2]
            idx_sb[v] = singles.tile([128, C * 8], i16, tag=f"idx_{v}", name=f"idxsb_{v}")
            nc.sync.dma_start(out=idx_sb[v][:], in_=idx_in[v][:])

        # per-table-buffer state for manual collective/gather dep tracking
        # (custom-DMA APs over DRAM pool tiles are not reliably dep-tracked)
        tabst = [{"ag": None, "preps": []} for _ in range(2)]
        slice_ag = [None, None]   # last AllGather reading tab_slices[i]

        def tab_phase(src_kind, vsrc_l, Wn, ts):
            """tab_slices[ts] = cast_bf16(src @ W) for own rows.
            src 'x': xT input; src 'f': fT_sl[vsrc_l] (both [feat, node])."""
            war = slice_ag[ts]
            for p in range(NPOS):
                cols = slice(p * 128, (p + 1) * 128)
                t_fn = pool.tile([128, 128], bf16, tag="tabin")
                if src_kind == "x":
                    ld = nc.sync.dma_start(out=t_fn[:], in_=xT_in[:, cols])
                else:
                    ld = nc.sync.dma_start(out=t_fn[:], in_=fT_sl[vsrc_l][:, cols])
                pm = psB.tile([128, 128], f32, tag="tabps")
                nc.tensor.matmul(pm[:], lhsT=W_sb[Wn][:], rhs=t_fn[:], start=True, stop=True)
                tmid = pool.tile([128, 128], f32, tag="tmid")
                nc.scalar.copy(out=tmid[:], in_=pm[:])
                ptr2 = psB.tile([128, 128], f32, tag="tabps2")
                nc.tensor.transpose(out=ptr2[:], in_=tmid[:], identity=identf[:])
                tb = pool.tile([128, 128], bf16, tag="tbf")
                nc.vector.tensor_copy(out=tb[:], in_=ptr2[:])
                st = nc.sync.dma_start(out=tab_slices[ts][p * 128:(p + 1) * 128, :], in_=tb[:])
                if war is not None:
                    add_dep_helper(st.ins, war.ins, reason="tab_slice WAR")
                yield st

        def allgather_table(tab_stores, ts, buf):
            ag = nc.gpsimd.collective_compute(
                "AllGather", mybir.AluOpType.bypass,
                replica_groups=[list(range(NCORES))],
                ins=[tab_slices[ts][:]], outs=[tables[buf][:]],
            )
            for st in tab_stores:
                add_dep_helper(ag.ins, st.ins, reason="tab_slice RAW")
            for g in tabst[buf]["preps"]:
                add_dep_helper(ag.ins, g.ins, reason="table WAR")
            tabst[buf] = {"ag": ag, "preps": []}
            slice_ag[ts] = ag

        def agg_phase(v, Wn, l_out, buf):
            NLO, NHI, C = meta[v]
            ag = tabst[buf]["ag"]
            sgs = [list(range(s, min(s + SG, NPOS))) for s in range(0, NPOS, SG)]
            chunk_base = 0  # global chunk counter
            idx_col = 0     # column offset into idx_sb (units of 16 idxs)
            for sg in sgs:
                nlo = int(sum(NLO[p] for p in sg))
                nhi = int(sum(NHI[p] for p in sg))
                nch_sg = nlo + nhi
                # stream this supergroup's selector chunks (contiguous)
                selsb = selp.tile([128, max(nch_sg, 1) * 128], bf16, tag="sel")
                nc.sync.dma_start(
                    out=selsb[:],
                    in_=sel_in[v][:, chunk_base * 128:(chunk_base + max(nch_sg, 1)) * 128])
                glo = gpo.tile([128, max(nlo, 1), 128], bf16, tag="glo")
                ghi = gpo.tile([128, max(nhi, 1), 128], bf16, tag="ghi")
                GMAXC = 8  # chunks per dma_gather (1024 idxs; >=2048 hangs SWDGE)
                for half_i, (nh, gt, lohi) in enumerate(
                        (((nlo, glo, (0, HALF)), (nhi, ghi, (HALF, NPAD))))):
                    for g0 in range(0, nh, GMAXC):
                        gn = min(GMAXC, nh - g0)
                        gi = nc.gpsimd.dma_gather(
                            out_ap=gt[:, g0:g0 + gn, :],
                            in_ap=tables[buf][lohi[0]:lohi[1], :],
                            idxs_ap=idx_sb[v][:, idx_col:idx_col + gn * 8],
                            num_idxs=gn * 128, num_idxs_reg=gn * 128, elem_size=128,
                        )
                        add_dep_helper(gi.ins, ag.ins, reason="table RAW")
                        tabst[buf]["preps"].append(gi)
                        idx_col += gn * 8
                # chunk order in sel array: [lo(p0)..lo(pk)] then [hi(p0)..hi(pk)]
                lo_off, off = {}, 0
                for p in sg:
                    lo_off[p] = off
                    off += int(NLO[p])
                hi_off, off = {}, 0
                for p in sg:
                    hi_off[p] = off
                    off += int(NHI[p])
                for p in sg:
                    ps = psA.tile([128, 128], f32, tag="agg")
                    nch = int(NLO[p] + NHI[p])
                    ci = 0
                    for k in range(int(NLO[p])):
                        cg = lo_off[p] + k  # sel col block within supergroup
                        nc.tensor.matmul(ps[:], lhsT=glo[:, lo_off[p] + k, :],
                                         rhs=selsb[:, cg * 128:(cg + 1) * 128],
                                         start=(ci == 0), stop=(ci == nch - 1))
                        ci += 1
                    for k in range(int(NHI[p])):
                        cg = nlo + hi_off[p] + k
                        nc.tensor.matmul(ps[:], lhsT=ghi[:, hi_off[p] + k, :],
                                         rhs=selsb[:, cg * 128:(cg + 1) * 128],
                                         start=(ci == 0), stop=(ci == nch - 1))
                        ci += 1
                    # postprocess: fT = relu(agg + b), fused row-sum for pooling
                    ft = fpo.tile([128, 128], bf16, tag="ftile")
                    racc = fpo.tile([128, 1], f32, tag="racc")
                    nc.scalar.activation(
                        out=ft[:], in_=ps[:], func=mybir.ActivationFunctionType.Relu,
                        bias=bb_sb[Wn][:, 0:1], accum_out=racc[:])
                    nc.vector.tensor_tensor(out=pooled_acc[:, l_out:l_out + 1],
                                            in0=pooled_acc[:, l_out:l_out + 1],
                                            in1=racc[:], op=mybir.AluOpType.add)
                    nc.scalar.dma_start(out=fT_sl[l_out][:, p * 128:(p + 1) * 128],
                                        in_=ft[:])
                chunk_base += nch_sg

        scope = os.environ.get("KERNEL_SCOPE", "full")
        if scope == "full":
            # schedule: tab f1, AG f1, tab s1, AG s1, agg f1, tab g1, AG g1,
            # agg s1, tab f2, AG f2, agg g1, ... so each AllGather overlaps
            # the previous layer's aggregation (ping-pong table buffers).
            plans = []
            for i, (v, ln) in enumerate(LAYERS):
                nm = f"{v}{ln}"
                src = ("x", None) if ln == 1 else ("f", 2 * VIEWS.index(v))
                plans.append({"v": v, "nm": nm, "src": src, "l_out": 2 * VIEWS.index(v) + ln - 1,
                              "buf": i % 2, "ts": i % 2})

            def do_tab(i):
                p = plans[i]
                stores = list(tab_phase(p["src"][0], p["src"][1], p["nm"], p["ts"]))
                allgather_table(stores, p["ts"], p["buf"])

            def do_agg(i):
                p = plans[i]
                agg_phase(p["v"], p["nm"], p["l_out"], p["buf"])

            do_tab(0)
            do_tab(1)
            do_agg(0)
            do_tab(2)
            do_agg(1)
            do_tab(3)
            do_agg(2)
            do_tab(4)
            do_agg(3)
            do_tab(5)
            do_agg(4)
            do_agg(5)
        else:
            plans = [{"src": ("x", None), "nm": "f1", "ts": 0, "buf": 0, "v": "f",
                      "l_out": 0}]
            stores = list(tab_phase("x", None, "f1", 0))
            if scope in ("tabag", "agg1", "f1out"):
                allgather_table(stores, 0, 0)
            if scope in ("agg1", "f1out"):
                agg_phase("f", "f1", 0, 0)

        # ---- pooled -> SE attention scalars ----
        pool_red = singles.tile([128, 6], f32)
        nc.gpsimd.partition_all_reduce(pool_red[:], pooled_acc[:], 128,
                                       bass_isa.ReduceOp.add)
        nc.sync.dma_start(out=pool6_in[:], in_=pool_red[0:1, 0:6])
        nc.gpsimd.collective_compute(
            "AllReduce", mybir.AluOpType.add,
            replica_groups=[list(range(NCORES))],
            ins=[pool6_in[:]], outs=[pool6_out[:]],
        )
        pvec2 = singles.tile([6, 1], f32)
        nc.sync.dma_start(out=pvec2[:], in_=pool6_out[:])
        corr_sb = singles.tile([6, 1], f32)
        nc.sync.dma_start(out=corr_sb[:], in_=corr_in.unsqueeze(1))
        # remove pad-column relu(bias) pollution, then mean
        nc.vector.tensor_tensor(out=pvec2[:], in0=pvec2[:], in1=corr_sb[:],
                                op=mybir.AluOpType.subtract)
        nc.vector.tensor_scalar_mul(pvec2[:], pvec2[:], 1.0 / (N * FM))
        fc1wT = singles.tile([6, 30], f32)
        nc.sync.dma_start(out=fc1wT[:], in_=fc1wT_in[:])
        fc1b = singles.tile([30, 1], f32)
        nc.sync.dma_start(out=fc1b[:], in_=fc1b_in.unsqueeze(1))
        fc2wT = singles.tile([30, 6], f32)
        nc.sync.dma_start(out=fc2wT[:], in_=fc2wT_in[:])
        fc2b = singles.tile([6, 1], f32)
        nc.sync.dma_start(out=fc2b[:], in_=fc2b_in.unsqueeze(1))
        pz1 = psB.tile([30, 1], f32, tag="tabps")
        nc.tensor.matmul(pz1[:], lhsT=fc1wT[:], rhs=pvec2[:], start=True, stop=True)
        z1 = singles.tile([30, 1], f32)
        nc.vector.tensor_tensor(out=z1[:], in0=pz1[:], in1=fc1b[:], op=mybir.AluOpType.add)
        nc.vector.tensor_scalar_max(z1[:], z1[:], 0.0)
        pz2 = psB.tile([6, 1], f32, tag="tabps")
        nc.tensor.matmul(pz2[:], lhsT=fc2wT[:], rhs=z1[:], start=True, stop=True)
        z2 = singles.tile([6, 1], f32)
        nc.vector.tensor_tensor(out=z2[:], in0=pz2[:], in1=fc2b[:], op=mybir.AluOpType.add)
        av = singles.tile([6, 1], f32)
        nc.scalar.activation(out=av[:], in_=z2[:], func=mybir.ActivationFunctionType.Sigmoid)
        nc.sync.dma_start(out=a_scr[:], in_=av[:, 0])
        a_b = singles.tile([128, 6], f32)
        nc.gpsimd.dma_start(out=a_b[:], in_=a_scr[:].partition_broadcast(128))
        cnnw_b = singles.tile([128, 6], f32)
        nc.gpsimd.dma_start(out=cnnw_b[:], in_=cnnw_in.partition_broadcast(128))
        cnnb_b = singles.tile([128, 1], f32)
        nc.gpsimd.dma_start(out=cnnb_b[:], in_=cnnb_in.partition_broadcast(128))

        # ---- final combine: outT = sum_l cnnw_l * relu(a_l * fT_l) + cnn_b ----
        if scope == "f1out":
            for p in range(NPOS):
                cols = slice(p * 128, (p + 1) * 128)
                fl0 = fpo.tile([128, 128], bf16, tag="fin", name=f"fl0_{p}")
                nc.sync.dma_start(out=fl0[:], in_=fT_sl[0][:, cols])
                fo = fpo.tile([128, 128], f32, tag="ftmp", name=f"fo_{p}")
                nc.vector.tensor_copy(out=fo[:], in_=fl0[:])
                nc.sync.dma_start(out=out_d[:, cols], in_=fo[:])
        nlayers = 6 if scope == "full" else 1
        for p in range(NPOS) if scope != "f1out" else []:
            cols = slice(p * 128, (p + 1) * 128)
            acc = fpo.tile([128, 128], f32, tag="facc")
            for l in range(nlayers):
                fl = fpo.tile([128, 128], bf16, tag="fin")
                nc.sync.dma_start(out=fl[:], in_=fT_sl[l][:, cols])
                t = fpo.tile([128, 128], f32, tag="ftmp")
                nc.scalar.activation(out=t[:], in_=fl[:],
                                     func=mybir.ActivationFunctionType.Relu,
                                     scale=a_b[:, l:l + 1])
                if l == 0:
                    nc.vector.tensor_scalar_mul(acc[:], t[:], cnnw_b[:, 0:1])
                else:
                    nc.vector.tensor_scalar_mul(t[:], t[:], cnnw_b[:, l:l + 1])
                    nc.vector.tensor_tensor(out=acc[:], in0=acc[:], in1=t[:],
                                            op=mybir.AluOpType.add)
            nc.vector.tensor_scalar_add(acc[:], acc[:], cnnb_b[:, 0:1])
            nc.sync.dma_start(out=out_d[:, cols], in_=acc[:])

    nc.compile()
    _split_multiwaits(nc)
    return nc


def kernel(**inputs):
    global _last_exec_time_ns
    inputs = {k: np.asarray(v) for k, v in inputs.items()}

    meta = {}
    perview = {}
    for v in VIEWS:
        idx_arrs, sel_arrs, NLO, NHI = _prep_view(
            inputs[f"edges_{v}"].astype(np.int64), inputs[f"ew_{v}"])
        meta[v] = (NLO, NHI, int((NLO + NHI).sum()))
        perview[v] = (idx_arrs, sel_arrs)

    nc = _build(meta)

    xT = inputs["x_m"].T.astype(np.float32)  # [128, N]
    xT_pad = np.zeros((128, NPAD), np.float32)
    xT_pad[:, :N] = xT
    xT_pad = xT_pad.astype(bfnp)
    ident_np = np.eye(128, dtype=bfnp)
    # pad dst columns (node ids >= N, all on core 7) read relu(bias) into the
    # pooled sum; precompute the exact pollution per layer and subtract it.
    npad_cols = NPAD - N
    corr = np.array(
        [npad_cols * np.maximum(inputs[f"b_{nm}"].astype(np.float64), 0).sum()
         for nm in ["f1", "f2", "s1", "s2", "g1", "g2"]], np.float32)

    in_maps = []
    for c in range(NCORES):
        m = {
            "xT_slice": np.ascontiguousarray(xT_pad[:, c * RPC:(c + 1) * RPC]),
            "ident": ident_np,
            "fc1wT": inputs["fc1_w"].T.astype(np.float32).copy(),
            "fc1b": inputs["fc1_b"].astype(np.float32),
            "fc2wT": inputs["fc2_w"].T.astype(np.float32).copy(),
            "fc2b": inputs["fc2_b"].astype(np.float32),
            "cnnw": inputs["cnn_w"].astype(np.float32),
            "cnnb": inputs["cnn_b"].astype(np.float32),
            "corr": corr,
        }
        for nm in ["f1", "f2", "s1", "s2", "g1", "g2"]:
            m[f"W_{nm}"] = inputs[f"W_{nm}"].astype(bfnp)
            m[f"b_{nm}"] = inputs[f"b_{nm}"].astype(np.float32)
        for v in VIEWS:
            idx_arrs, sel_arrs = perview[v]
            m[f"idx_{v}"] = idx_arrs[c]
            m[f"sel_{v}"] = sel_arrs[c]
        in_maps.append(m)

    trace = os.environ.get("KERNEL_TRACE", "0") == "1"
    kw = {}
    if trace:
        td = os.environ.get("KERNEL_TRACE_DIR")
        if td:
            os.makedirs(td, exist_ok=True)
            kw["tmpdir"] = td
    res = run_bass_kernel_spmd(nc, in_maps, list(range(NCORES)), trace=trace, **kw)
    _last_exec_time_ns = res.exec_time_ns
    outT = np.concatenate([res.results[c]["out_slice"] for c in range(NCORES)], axis=1)
    return np.ascontiguousarray(outT.T[:N]).astype(np.float32)



# revision 13
# speedup vs baseline: 1.4209x; 1.0307x over previous
"""GCN message-passing kernel for 8 Trainium2 NeuronCores.

Strategy: shard destination nodes across cores (6272 rows/core). Each core
aggregates all edges targeting its rows by gathering source rows from a
replicated bf16 node-feature table (SWDGE dma_gather, prepare_only +
trigger_dma so the Pool engine pipelines descriptor-gen with transfers) and
contracting each 128-edge chunk against a host-precomputed one-hot selector
(streamed from HBM) on the PE array. The aggregation runs transposed
(psum[feat, dst]) so bias+relu+row-sum fuse into one Activation-engine op.
Layer tables ping-pong between two DRAM buffers so each AllGather overlaps
the previous layer's aggregation. SE attention + 1x1 conv are tiny and
replicated; the final output is produced transposed and fixed up on host.
"""
import os
import sys

sys.path.insert(0, "/opt/trn_rl_repo")

from contextlib import ExitStack

import ml_dtypes
import numpy as np

import concourse.bacc as bacc
import concourse.tile as tile
from concourse.tile import add_dep_helper
from concourse import bass_isa, mybir
from concourse.bass_utils import run_bass_kernel_spmd

N = 50000
FM = 128
E = 800000
NCORES = 8
NPOS = 49                  # 128-row tiles per core
RPC = NPOS * 128           # 6272 rows per core
NPAD = NCORES * RPC        # 50176
HALF = NPAD // 2           # 25088 (int16 gather index limit per table half)
SG = 4                     # positions per gather supergroup
VIEWS = ("f", "s", "g")
LAYERS = [("f", 1), ("s", 1), ("g", 1), ("f", 2), ("s", 2), ("g", 2)]

f32 = mybir.dt.float32
bf16 = mybir.dt.bfloat16
i16 = mybir.dt.int16
bfnp = ml_dtypes.bfloat16

_last_exec_time_ns = None


def _split_multiwaits(nc):
    """This walrus build accepts only ONE sync-wait per instruction; split
    extras into preceding same-engine single-wait NoOps (sequencer executes
    waits in program order, so semantics are preserved)."""
    n = 0
    for fn in nc.m.functions:
        for bb in fn.blocks:
            newlist = []
            for inst in bb.instructions:
                si = inst.sync_info
                if si is not None and len(si.on_wait) > 1:
                    waits = list(si.on_wait)
                    for w in waits[:-1]:
                        nop = mybir.InstNoOp(name=f"WSPL-{nc.next_id()}", ins=[], outs=[])
                        nop.engine = inst.engine
                        nop.sync_info = mybir.SyncInfo(on_wait=[w], on_update=[])
                        newlist.append(nop)
                        n += 1
                    si.on_wait = [waits[-1]]
                newlist.append(inst)
            bb.instructions = newlist
    return n


def _prep_view(edges, ew):
    """Host edge preprocessing for one view: append self-loops, compute the
    symmetric GCN normalization, shard by destination across cores, group by
    (dst tile, src half), pad runs to 128-edge chunks (uniform across cores).

    Returns (idx_arrs, sel_arrs, NLO, NHI): per-core SWDGE index arrays and
    precomputed one-hot selector chunks ([128 edge-slot partitions, C*128
    dst columns], bf16, selector value = the edge's GCN norm weight)."""
    src = np.concatenate([edges[0], np.arange(N, dtype=np.int64)])
    dst = np.concatenate([edges[1], np.arange(N, dtype=np.int64)])
    w = np.concatenate([ew.astype(np.float64), np.ones(N)])
    deg = np.bincount(dst, weights=w, minlength=N)
    dis = 1.0 / np.sqrt(deg)
    norm = (dis[src] * w * dis[dst]).astype(np.float32)

    core = dst // RPC
    pos = (dst % RPC) // 128
    dstrel = (dst % 128).astype(np.int64)
    half = (src >= HALF).astype(np.int64)
    idx = (src - HALF * half).astype(np.int16)

    # counts[c, p, h]
    key = (core * NPOS + pos) * 2 + half
    counts = np.bincount(key, minlength=NCORES * NPOS * 2).reshape(NCORES, NPOS, 2)
    chunks = -(-counts // 128)  # ceil
    NLO = chunks[:, :, 0].max(axis=0)
    NHI = chunks[:, :, 1].max(axis=0)

    order = np.lexsort((half, pos, core))
    norm_s, dstrel_s, idx_s, key_s = (
        norm[order], dstrel[order], idx[order], key[order])
    starts = np.searchsorted(key_s, np.arange(NCORES * NPOS * 2))
    ends = np.searchsorted(key_s, np.arange(NCORES * NPOS * 2), side="right")

    C = int((NLO + NHI).sum())
    idx_arrs, sel_arrs = [], []
    sgs = [list(range(s, min(s + SG, NPOS))) for s in range(0, NPOS, SG)]
    for c in range(NCORES):
        idx_a = np.zeros(C * 128, np.int16)
        dr_a = np.zeros(C * 128, np.int64)
        w_a = np.zeros(C * 128, np.float32)
        off = 0
        for sg in sgs:
            for h in range(2):
                for p in sg:
                    k = (c * NPOS + p) * 2 + h
                    s0, e0 = starts[k], ends[k]
                    n = e0 - s0
                    nch = (NLO if h == 0 else NHI)[p]
                    idx_a[off:off + n] = idx_s[s0:e0]
                    dr_a[off:off + n] = dstrel_s[s0:e0]
                    w_a[off:off + n] = norm_s[s0:e0]
                    off += nch * 128
        assert off == C * 128
        # device layouts
        idx_wrapped = np.tile(idx_a.reshape(-1, 16).T, (8, 1)).copy()  # [128, C*8]
        sel_flat = np.zeros((C * 128, 128), np.float32)
        sel_flat[np.arange(C * 128), dr_a] = w_a
        sel_dev = np.ascontiguousarray(
            sel_flat.reshape(C, 128, 128).transpose(1, 0, 2).reshape(128, C * 128)
        ).astype(bfnp)
        idx_arrs.append(idx_wrapped)
        sel_arrs.append(sel_dev)
    return idx_arrs, sel_arrs, NLO.astype(int), NHI.astype(int)


def _build(meta):
    """Build the SPMD program. meta[v] = (NLO, NHI, C) per view."""
    nc = bacc.Bacc("TRN2", target_bir_lowering=False, debug=False,
                   num_devices=NCORES)

    # ---- I/O ----
    xfull_in = nc.dram_tensor("x_full", [NPAD, FM], bf16, kind="ExternalInput").ap()
    W_in, b_in = {}, {}
    for nm in ["f1", "f2", "s1", "s2", "g1", "g2"]:
        W_in[nm] = nc.dram_tensor(f"W_{nm}", [FM, FM], bf16, kind="ExternalInput").ap()
        b_in[nm] = nc.dram_tensor(f"b_{nm}", [FM], f32, kind="ExternalInput").ap()
    idx_in, sel_in = {}, {}
    for v in VIEWS:
        C = meta[v][2]
        idx_in[v] = nc.dram_tensor(f"idx_{v}", [128, C * 8], i16, kind="ExternalInput").ap()
        sel_in[v] = nc.dram_tensor(f"sel_{v}", [128, C * 128], bf16, kind="ExternalInput").ap()
    ident_in = nc.dram_tensor("ident", [128, 128], bf16, kind="ExternalInput").ap()
    fc1wT_in = nc.dram_tensor("fc1wT", [6, 30], f32, kind="ExternalInput").ap()
    fc1b_in = nc.dram_tensor("fc1b", [30], f32, kind="ExternalInput").ap()
    fc2wT_in = nc.dram_tensor("fc2wT", [30, 6], f32, kind="ExternalInput").ap()
    fc2b_in = nc.dram_tensor("fc2b", [6], f32, kind="ExternalInput").ap()
    cnnw_in = nc.dram_tensor("cnnw", [6], f32, kind="ExternalInput").ap()
    cnnb_in = nc.dram_tensor("cnnb", [1], f32, kind="ExternalInput").ap()
    corr_in = nc.dram_tensor("corr", [6], f32, kind="ExternalInput").ap()
    out_d = nc.dram_tensor("out_slice", [FM, RPC], f32, kind="ExternalOutput").ap()

    dma_sem = nc.alloc_semaphore("gather_dma")

    with tile.TileContext(nc) as tc, ExitStack() as ctx:
        singles = ctx.enter_context(tc.tile_pool(name="singles", bufs=1))
        pool = ctx.enter_context(tc.tile_pool(name="pool", bufs=3))
        selp = ctx.enter_context(tc.tile_pool(name="selp", bufs=2))
        gpo = ctx.enter_context(tc.tile_pool(name="gpo", bufs=2))
        fpo = ctx.enter_context(tc.tile_pool(name="fpo", bufs=4))
        psA = ctx.enter_context(tc.tile_pool(name="psA", bufs=4, space="PSUM"))
        psB = ctx.enter_context(tc.tile_pool(name="psB", bufs=2, space="PSUM"))
        dram = ctx.enter_context(tc.tile_pool(name="dram", bufs=1, space="DRAM"))

        tables = [dram.tile([NPAD, FM], bf16, name=f"table{i}") for i in range(3)]
        tab_slices = [dram.tile([RPC, FM], bf16, name=f"tab_slice{i}") for i in range(2)]
        fT_sl = [dram.tile([128, RPC], bf16, name=f"fT_sl{i}") for i in range(6)]
        pool6_in = dram.tile([6, 1], f32, name="pool6_in")
        pool6_out = dram.tile([6, 1], f32, name="pool6_out")
        a_scr = dram.tile([6], f32, name="a_scr")

        # ---- constants ----
        ident = singles.tile([128, 128], bf16)
        nc.sync.dma_start(out=ident[:], in_=ident_in[:])
        identf = singles.tile([128, 128], f32)
        nc.vector.tensor_copy(out=identf[:], in_=ident[:])
        W_sb, bb_sb = {}, {}
        for nm in ["f1", "f2", "s1", "s2", "g1", "g2"]:
            W_sb[nm] = singles.tile([FM, FM], bf16, tag=f"W_{nm}", name=f"Wsb_{nm}")
            nc.sync.dma_start(out=W_sb[nm][:], in_=W_in[nm][:])
            bb_sb[nm] = singles.tile([FM, 1], f32, tag=f"bb_{nm}", name=f"bbsb_{nm}")
            nc.sync.dma_start(out=bb_sb[nm][:], in_=b_in[nm].unsqueeze(1))
        pooled_acc = singles.tile([128, 6], f32)
        nc.vector.memset(pooled_acc[:], 0.0)

        idx_sb = {}
        for v in VIEWS:
            C = meta[v][2]
            idx_sb[v] = singles.tile([128, C * 8], i16, tag=f"idx_{v}", name=f"idxsb_{v}")
            nc.sync.dma_start(out=idx_sb[v][:], in_=idx_in[v][:])

        # per-table-buffer state for manual collective/gather dep tracking
        # (custom-DMA APs over DRAM pool tiles are not reliably dep-tracked)
        tabst = [{"ag": None, "preps": []} for _ in range(3)]
        slice_ag = [None, None]   # last AllGather reading tab_slices[i]

        def tab_phase(src_kind, vsrc_l, Wn, ts):
            """tab_slices[ts] = cast_bf16(src @ W) for own rows.
            src 'x': xT input; src 'f': fT_sl[vsrc_l] (both [feat, node])."""
            war = slice_ag[ts]
            for p in range(NPOS):
                cols = slice(p * 128, (p + 1) * 128)
                t_fn = pool.tile([128, 128], bf16, tag="tabin")
                assert src_kind == "f"
                ld = nc.sync.dma_start(out=t_fn[:], in_=fT_sl[vsrc_l][:, cols])
                pm = psB.tile([128, 128], f32, tag="tabps")
                nc.tensor.matmul(pm[:], lhsT=W_sb[Wn][:], rhs=t_fn[:], start=True, stop=True)
                tmid = pool.tile([128, 128], f32, tag="tmid")
                nc.scalar.copy(out=tmid[:], in_=pm[:])
                ptr2 = psB.tile([128, 128], f32, tag="tabps2")
                nc.tensor.transpose(out=ptr2[:], in_=tmid[:], identity=identf[:])
                tb = pool.tile([128, 128], bf16, tag="tbf")
                nc.vector.tensor_copy(out=tb[:], in_=ptr2[:])
                st = nc.sync.dma_start(out=tab_slices[ts][p * 128:(p + 1) * 128, :], in_=tb[:])
                if war is not None:
                    add_dep_helper(st.ins, war.ins, reason="tab_slice WAR")
                yield st

        def allgather_table(tab_stores, ts, buf):
            ag = nc.gpsimd.collective_compute(
                "AllGather", mybir.AluOpType.bypass,
                replica_groups=[list(range(NCORES))],
                ins=[tab_slices[ts][:]], outs=[tables[buf][:]],
            )
            for st in tab_stores:
                add_dep_helper(ag.ins, st.ins, reason="tab_slice RAW")
            for g in tabst[buf]["preps"]:
                add_dep_helper(ag.ins, g.ins, reason="table WAR")
            tabst[buf] = {"ag": ag, "preps": []}
            slice_ag[ts] = ag

        def agg_phase(v, Wn, l_out, buf, from_x=False):
            NLO, NHI, C = meta[v]
            ag = None if from_x else tabst[buf]["ag"]
            sgs = [list(range(s, min(s + SG, NPOS))) for s in range(0, NPOS, SG)]
            chunk_base = 0  # global chunk counter
            idx_col = 0     # column offset into idx_sb (units of 16 idxs)
            for sg in sgs:
                nlo = int(sum(NLO[p] for p in sg))
                nhi = int(sum(NHI[p] for p in sg))
                nch_sg = nlo + nhi
                # stream this supergroup's selector chunks (contiguous)
                selsb = selp.tile([128, max(nch_sg, 1) * 128], bf16, tag="sel")
                nc.sync.dma_start(
                    out=selsb[:],
                    in_=sel_in[v][:, chunk_base * 128:(chunk_base + max(nch_sg, 1)) * 128])
                glo = gpo.tile([128, max(nlo, 1), 128], bf16, tag="glo")
                ghi = gpo.tile([128, max(nhi, 1), 128], bf16, tag="ghi")
                GMAXC = int(os.environ.get("KERNEL_GMAXC", "8"))  # chunks per dma_gather (1024 idxs default; >=2048 hangs SWDGE)
                for half_i, (nh, gt, lohi) in enumerate(
                        (((nlo, glo, (0, HALF)), (nhi, ghi, (HALF, NPAD))))):
                    for g0 in range(0, nh, GMAXC):
                        gn = min(GMAXC, nh - g0)
                        src_tab = (xfull_in if from_x else tables[buf])
                        gi = nc.gpsimd.dma_gather(
                            out_ap=gt[:, g0:g0 + gn, :],
                            in_ap=src_tab[lohi[0]:lohi[1], :],
                            idxs_ap=idx_sb[v][:, idx_col:idx_col + gn * 8],
                            num_idxs=gn * 128, num_idxs_reg=gn * 128, elem_size=128,
                        )
                        if not from_x:
                            add_dep_helper(gi.ins, ag.ins, reason="table RAW")
                            tabst[buf]["preps"].append(gi)
                        idx_col += gn * 8
                # chunk order in sel array: [lo(p0)..lo(pk)] then [hi(p0)..hi(pk)]
                lo_off, off = {}, 0
                for p in sg:
                    lo_off[p] = off
                    off += int(NLO[p])
                hi_off, off = {}, 0
                for p in sg:
                    hi_off[p] = off
                    off += int(NHI[p])
                for p in sg:
                    ps = psA.tile([128, 128], f32, tag="agg")
                    nch = int(NLO[p] + NHI[p])
                    ci = 0
                    for k in range(int(NLO[p])):
                        cg = lo_off[p] + k  # sel col block within supergroup
                        nc.tensor.matmul(ps[:], lhsT=glo[:, lo_off[p] + k, :],
                                         rhs=selsb[:, cg * 128:(cg + 1) * 128],
                                         start=(ci == 0), stop=(ci == nch - 1))
                        ci += 1
                    for k in range(int(NHI[p])):
                        cg = nlo + hi_off[p] + k
                        nc.tensor.matmul(ps[:], lhsT=ghi[:, hi_off[p] + k, :],
                                         rhs=selsb[:, cg * 128:(cg + 1) * 128],
                                         start=(ci == 0), stop=(ci == nch - 1))
                        ci += 1
                    if from_x:
                        # aggregate-then-transform: psum holds (A@x)^T [fi,dst];
                        # apply W on-PE before bias+relu.
                        aggT = fpo.tile([128, 128], bf16, tag="aggT")
                        nc.scalar.copy(out=aggT[:], in_=ps[:])
                        ps2 = psB.tile([128, 128], f32, tag="tabps")
                        nc.tensor.matmul(ps2[:], lhsT=W_sb[Wn][:], rhs=aggT[:],
                                         start=True, stop=True)
                        ps = ps2
                    # postprocess: fT = relu(agg + b), fused row-sum for pooling
                    ft = fpo.tile([128, 128], bf16, tag="ftile")
                    racc = fpo.tile([128, 1], f32, tag="racc")
                    nc.scalar.activation(
                        out=ft[:], in_=ps[:], func=mybir.ActivationFunctionType.Relu,
                        bias=bb_sb[Wn][:, 0:1], accum_out=racc[:])
                    nc.vector.tensor_tensor(out=pooled_acc[:, l_out:l_out + 1],
                                            in0=pooled_acc[:, l_out:l_out + 1],
                                            in1=racc[:], op=mybir.AluOpType.add)
                    nc.scalar.dma_start(out=fT_sl[l_out][:, p * 128:(p + 1) * 128],
                                        in_=ft[:])
                chunk_base += nch_sg

        scope = os.environ.get("KERNEL_SCOPE", "full")
        if scope == "full":
            # schedule: tab f1, AG f1, tab s1, AG s1, agg f1, tab g1, AG g1,
            # agg s1, tab f2, AG f2, agg g1, ... so each AllGather overlaps
            # the previous layer's aggregation (ping-pong table buffers).
            plans = []
            for i, (v, ln) in enumerate(LAYERS):
                nm = f"{v}{ln}"
                src = ("x", None) if ln == 1 else ("f", 2 * VIEWS.index(v))
                plans.append({"v": v, "nm": nm, "src": src, "l_out": 2 * VIEWS.index(v) + ln - 1,
                              "buf": VIEWS.index(v), "ts": i % 2})

            def do_tab(i):
                p = plans[i]
                stores = list(tab_phase(p["src"][0], p["src"][1], p["nm"], p["ts"]))
                allgather_table(stores, p["ts"], p["buf"])

            def do_agg(i, from_x=False):
                p = plans[i]
                agg_phase(p["v"], p["nm"], p["l_out"], p["buf"], from_x=from_x)

            # layer-1 aggregations gather raw x rows from the replicated input
            # table (no tab phase, no AllGather) and post-multiply by W; only
            # the second layers need table AllGathers.
            do_agg(0, from_x=True)
            do_tab(3)
            do_agg(1, from_x=True)
            do_tab(4)
            do_agg(2, from_x=True)
            do_tab(5)
            do_agg(3)
            do_agg(4)
            do_agg(5)
        else:
            plans = [{"src": ("x", None), "nm": "f1", "ts": 0, "buf": 0, "v": "f",
                      "l_out": 0}]
            if scope in ("agg1", "f1out"):
                agg_phase("f", "f1", 0, 0, from_x=True)

        # ---- pooled -> SE attention scalars ----
        pool_red = singles.tile([128, 6], f32)
        nc.gpsimd.partition_all_reduce(pool_red[:], pooled_acc[:], 128,
                                       bass_isa.ReduceOp.add)
        nc.sync.dma_start(out=pool6_in[:], in_=pool_red[0:1, 0:6])
        nc.gpsimd.collective_compute(
            "AllReduce", mybir.AluOpType.add,
            replica_groups=[list(range(NCORES))],
            ins=[pool6_in[:]], outs=[pool6_out[:]],
        )
        pvec2 = singles.tile([6, 1], f32)
        nc.sync.dma_start(out=pvec2[:], in_=pool6_out[:])
        corr_sb = singles.tile([6, 1], f32)
        nc.sync.dma_start(out=corr_sb[:], in_=corr_in.unsqueeze(1))
        # remove pad-column relu(bias) pollution, then mean
        nc.vector.tensor_tensor(out=pvec2[:], in0=pvec2[:], in1=corr_sb[:],
                                op=mybir.AluOpType.subtract)
        nc.vector.tensor_scalar_mul(pvec2[:], pvec2[:], 1.0 / (N * FM))
        fc1wT = singles.tile([6, 30], f32)
        nc.sync.dma_start(out=fc1wT[:], in_=fc1wT_in[:])
        fc1b = singles.tile([30, 1], f32)
        nc.sync.dma_start(out=fc1b[:], in_=fc1b_in.unsqueeze(1))
        fc2wT = singles.tile([30, 6], f32)
        nc.sync.dma_start(out=fc2wT[:], in_=fc2wT_in[:])
        fc2b = singles.tile([6, 1], f32)
        nc.sync.dma_start(out=fc2b[:], in_=fc2b_in.unsqueeze(1))
        pz1 = psB.tile([30, 1], f32, tag="tabps")
        nc.tensor.matmul(pz1[:], lhsT=fc1wT[:], rhs=pvec2[:], start=True, stop=True)
        z1 = singles.tile([30, 1], f32)
        nc.vector.tensor_tensor(out=z1[:], in0=pz1[:], in1=fc1b[:], op=mybir.AluOpType.add)
        nc.vector.tensor_scalar_max(z1[:], z1[:], 0.0)
        pz2 = psB.tile([6, 1], f32, tag="tabps")
        nc.tensor.matmul(pz2[:], lhsT=fc2wT[:], rhs=z1[:], start=True, stop=True)
        z2 = singles.tile([6, 1], f32)
        nc.vector.tensor_tensor(out=z2[:], in0=pz2[:], in1=fc2b[:], op=mybir.AluOpType.add)
        av = singles.tile([6, 1], f32)
        nc.scalar.activation(out=av[:], in_=z2[:], func=mybir.ActivationFunctionType.Sigmoid)
        nc.sync.dma_start(out=a_scr[:], in_=av[:, 0])
        a_b = singles.tile([128, 6], f32)
        nc.gpsimd.dma_start(out=a_b[:], in_=a_scr[:].partition_broadcast(128))
        cnnw_b = singles.tile([128, 6], f32)
        nc.gpsimd.dma_start(out=cnnw_b[:], in_=cnnw_in.partition_broadcast(128))
        cnnb_b = singles.tile([128, 1], f32)
        nc.gpsimd.dma_start(out=cnnb_b[:], in_=cnnb_in.partition_broadcast(128))

        # ---- final combine: outT = sum_l cnnw_l * relu(a_l * fT_l) + cnn_b ----
        if scope == "f1out":
            for p in range(NPOS):
                cols = slice(p * 128, (p + 1) * 128)
                fl0 = fpo.tile([128, 128], bf16, tag="fin", name=f"fl0_{p}")
                nc.sync.dma_start(out=fl0[:], in_=fT_sl[0][:, cols])
                fo = fpo.tile([128, 128], f32, tag="ftmp", name=f"fo_{p}")
                nc.vector.tensor_copy(out=fo[:], in_=fl0[:])
                nc.sync.dma_start(out=out_d[:, cols], in_=fo[:])
        nlayers = 6 if scope == "full" else 1
        for p in range(NPOS) if scope != "f1out" else []:
            cols = slice(p * 128, (p + 1) * 128)
            acc = fpo.tile([128, 128], f32, tag="facc")
            for l in range(nlayers):
                fl = fpo.tile([128, 128], bf16, tag="fin")
                nc.sync.dma_start(out=fl[:], in_=fT_sl[l][:, cols])
                t = fpo.tile([128, 128], f32, tag="ftmp")
                nc.scalar.activation(out=t[:], in_=fl[:],
                                     func=mybir.ActivationFunctionType.Relu,
                                     scale=a_b[:, l:l + 1])
                if l == 0:
                    nc.vector.tensor_scalar_mul(acc[:], t[:], cnnw_b[:, 0:1])
                else:
                    nc.vector.tensor_scalar_mul(t[:], t[:], cnnw_b[:, l:l + 1])
                    nc.vector.tensor_tensor(out=acc[:], in0=acc[:], in1=t[:],
                                            op=mybir.AluOpType.add)
            nc.vector.tensor_scalar_add(acc[:], acc[:], cnnb_b[:, 0:1])
            nc.sync.dma_start(out=out_d[:, cols], in_=acc[:])

    nc.compile()
    _split_multiwaits(nc)
    return nc


def kernel(**inputs):
    global _last_exec_time_ns
    inputs = {k: np.asarray(v) for k, v in inputs.items()}

    meta = {}
    perview = {}
    for v in VIEWS:
        idx_arrs, sel_arrs, NLO, NHI = _prep_view(
            inputs[f"edges_{v}"].astype(np.int64), inputs[f"ew_{v}"])
        meta[v] = (NLO, NHI, int((NLO + NHI).sum()))
        perview[v] = (idx_arrs, sel_arrs)

    nc = _build(meta)

    x_pad = np.zeros((NPAD, FM), np.float32)
    x_pad[:N, :] = inputs["x_m"].astype(np.float32)
    x_pad = np.ascontiguousarray(x_pad).astype(bfnp)
    ident_np = np.eye(128, dtype=bfnp)
    # pad dst columns (node ids >= N, all on core 7) read relu(bias) into the
    # pooled sum; precompute the exact pollution per layer and subtract it.
    npad_cols = NPAD - N
    corr = np.array(
        [npad_cols * np.maximum(inputs[f"b_{nm}"].astype(np.float64), 0).sum()
         for nm in ["f1", "f2", "s1", "s2", "g1", "g2"]], np.float32)

    in_maps = []
    for c in range(NCORES):
        m = {
            "x_full": x_pad,
            "ident": ident_np,
            "fc1wT": inputs["fc1_w"].T.astype(np.float32).copy(),
            "fc1b": inputs["fc1_b"].astype(np.float32),
            "fc2wT": inputs["fc2_w"].T.astype(np.float32).copy(),
            "fc2b": inputs["fc2_b"].astype(np.float32),
            "cnnw": inputs["cnn_w"].astype(np.float32),
            "cnnb": inputs["cnn_b"].astype(np.float32),
            "corr": corr,
        }
        for nm in ["f1", "f2", "s1", "s2", "g1", "g2"]:
            m[f"W_{nm}"] = inputs[f"W_{nm}"].astype(bfnp)
            m[f"b_{nm}"] = inputs[f"b_{nm}"].astype(np.float32)
        for v in VIEWS:
            idx_arrs, sel_arrs = perview[v]
            m[f"idx_{v}"] = idx_arrs[c]
            m[f"sel_{v}"] = sel_arrs[c]
        in_maps.append(m)

    trace = os.environ.get("KERNEL_TRACE", "0") == "1"
    kw = {}
    if trace:
        td = os.environ.get("KERNEL_TRACE_DIR")
        if td:
            os.makedirs(td, exist_ok=True)
            kw["tmpdir"] = td
    res = run_bass_kernel_spmd(nc, in_maps, list(range(NCORES)), trace=trace, **kw)
    _last_exec_time_ns = res.exec_time_ns
    outT = np.concatenate([res.results[c]["out_slice"] for c in range(NCORES)], axis=1)
    return np.ascontiguousarray(outT.T[:N]).astype(np.float32)



# revision 14
# speedup vs baseline: 1.4245x; 1.0025x over previous
"""GCN message-passing kernel for 8 Trainium2 NeuronCores.

Strategy: shard destination nodes across cores (6272 rows/core). Each core
aggregates all edges targeting its rows by gathering source rows with
synchronous SWDGE dma_gather (1024 idxs/fire; >1024 hangs this ucode, and
every fire blocks the Pool engine ~8.5us regardless of mode, so sync issue
is optimal) and contracting each 128-edge chunk against a host-precomputed
one-hot selector (streamed from HBM) on the PE array. Layer-1 aggregations
use aggregate-then-transform (A@(xW) = (A@x)W): they gather raw x rows from
a replicated host-uploaded table and apply W on-PE afterwards, so no tab
phase or AllGather precedes them and the gather pipeline starts at t=0;
only the three layer-2 tables are computed+AllGathered (into 3 dedicated
DRAM buffers), overlapping the preceding aggregations. The aggregation runs
transposed (psum[feat, dst]) so bias+relu+row-sum fuse into one
Activation-engine op. SE attention + 1x1 conv are tiny and replicated; the
final output is produced transposed and fixed up on host.
"""
import os
import sys

sys.path.insert(0, "/opt/trn_rl_repo")

from contextlib import ExitStack

import ml_dtypes
import numpy as np

import concourse.bacc as bacc
import concourse.tile as tile
from concourse.tile import add_dep_helper
from concourse import bass_isa, mybir
from concourse.bass_utils import run_bass_kernel_spmd

N = 50000
FM = 128
E = 800000
NCORES = 8
NPOS = 49                  # 128-row tiles per core
RPC = NPOS * 128           # 6272 rows per core
NPAD = NCORES * RPC        # 50176
HALF = NPAD // 2           # 25088 (int16 gather index limit per table half)
SG = 4                     # positions per gather supergroup
VIEWS = ("f", "s", "g")
LAYERS = [("f", 1), ("s", 1), ("g", 1), ("f", 2), ("s", 2), ("g", 2)]

f32 = mybir.dt.float32
bf16 = mybir.dt.bfloat16
i16 = mybir.dt.int16
bfnp = ml_dtypes.bfloat16

_last_exec_time_ns = None


def _split_multiwaits(nc):
    """This walrus build accepts only ONE sync-wait per instruction; split
    extras into preceding same-engine single-wait NoOps (sequencer executes
    waits in program order, so semantics are preserved)."""
    n = 0
    for fn in nc.m.functions:
        for bb in fn.blocks:
            newlist = []
            for inst in bb.instructions:
                si = inst.sync_info
                if si is not None and len(si.on_wait) > 1:
                    waits = list(si.on_wait)
                    for w in waits[:-1]:
                        nop = mybir.InstNoOp(name=f"WSPL-{nc.next_id()}", ins=[], outs=[])
                        nop.engine = inst.engine
                        nop.sync_info = mybir.SyncInfo(on_wait=[w], on_update=[])
                        newlist.append(nop)
                        n += 1
                    si.on_wait = [waits[-1]]
                newlist.append(inst)
            bb.instructions = newlist
    return n


def _prep_view(edges, ew):
    """Host edge preprocessing for one view: append self-loops, compute the
    symmetric GCN normalization, shard by destination across cores, group by
    (dst tile, src half), pad runs to 128-edge chunks (uniform across cores).

    Returns (idx_arrs, sel_arrs, NLO, NHI): per-core SWDGE index arrays and
    precomputed one-hot selector chunks ([128 edge-slot partitions, C*128
    dst columns], bf16, selector value = the edge's GCN norm weight)."""
    src = np.concatenate([edges[0], np.arange(N, dtype=np.int64)])
    dst = np.concatenate([edges[1], np.arange(N, dtype=np.int64)])
    w = np.concatenate([ew.astype(np.float64), np.ones(N)])
    deg = np.bincount(dst, weights=w, minlength=N)
    dis = 1.0 / np.sqrt(deg)
    norm = (dis[src] * w * dis[dst]).astype(np.float32)

    core = dst // RPC
    pos = (dst % RPC) // 128
    dstrel = (dst % 128).astype(np.int64)
    half = (src >= HALF).astype(np.int64)
    idx = (src - HALF * half).astype(np.int16)

    # counts[c, p, h]
    key = (core * NPOS + pos) * 2 + half
    counts = np.bincount(key, minlength=NCORES * NPOS * 2).reshape(NCORES, NPOS, 2)
    chunks = -(-counts // 128)  # ceil
    NLO = chunks[:, :, 0].max(axis=0)
    NHI = chunks[:, :, 1].max(axis=0)

    order = np.lexsort((half, pos, core))
    norm_s, dstrel_s, idx_s, key_s = (
        norm[order], dstrel[order], idx[order], key[order])
    starts = np.searchsorted(key_s, np.arange(NCORES * NPOS * 2))
    ends = np.searchsorted(key_s, np.arange(NCORES * NPOS * 2), side="right")

    C = int((NLO + NHI).sum())
    idx_arrs, sel_arrs = [], []
    sgs = [list(range(s, min(s + SG, NPOS))) for s in range(0, NPOS, SG)]
    for c in range(NCORES):
        idx_a = np.zeros(C * 128, np.int16)
        dr_a = np.zeros(C * 128, np.int64)
        w_a = np.zeros(C * 128, np.float32)
        off = 0
        for sg in sgs:
            for h in range(2):
                for p in sg:
                    k = (c * NPOS + p) * 2 + h
                    s0, e0 = starts[k], ends[k]
                    n = e0 - s0
                    nch = (NLO if h == 0 else NHI)[p]
                    idx_a[off:off + n] = idx_s[s0:e0]
                    dr_a[off:off + n] = dstrel_s[s0:e0]
                    w_a[off:off + n] = norm_s[s0:e0]
                    off += nch * 128
        assert off == C * 128
        # device layouts
        idx_wrapped = np.tile(idx_a.reshape(-1, 16).T, (8, 1)).copy()  # [128, C*8]
        sel_flat = np.zeros((C * 128, 128), np.float32)
        sel_flat[np.arange(C * 128), dr_a] = w_a
        sel_dev = np.ascontiguousarray(
            sel_flat.reshape(C, 128, 128).transpose(1, 0, 2).reshape(128, C * 128)
        ).astype(bfnp)
        idx_arrs.append(idx_wrapped)
        sel_arrs.append(sel_dev)
    return idx_arrs, sel_arrs, NLO.astype(int), NHI.astype(int)


def _build(meta):
    """Build the SPMD program. meta[v] = (NLO, NHI, C) per view."""
    nc = bacc.Bacc("TRN2", target_bir_lowering=False, debug=False,
                   num_devices=NCORES)

    # ---- I/O ----
    xfull_in = nc.dram_tensor("x_full", [NPAD, FM], bf16, kind="ExternalInput").ap()
    W_in, b_in = {}, {}
    for nm in ["f1", "f2", "s1", "s2", "g1", "g2"]:
        W_in[nm] = nc.dram_tensor(f"W_{nm}", [FM, FM], bf16, kind="ExternalInput").ap()
        b_in[nm] = nc.dram_tensor(f"b_{nm}", [FM], f32, kind="ExternalInput").ap()
    idx_in, sel_in = {}, {}
    for v in VIEWS:
        C = meta[v][2]
        idx_in[v] = nc.dram_tensor(f"idx_{v}", [128, C * 8], i16, kind="ExternalInput").ap()
        sel_in[v] = nc.dram_tensor(f"sel_{v}", [128, C * 128], bf16, kind="ExternalInput").ap()
    ident_in = nc.dram_tensor("ident", [128, 128], bf16, kind="ExternalInput").ap()
    fc1wT_in = nc.dram_tensor("fc1wT", [6, 30], f32, kind="ExternalInput").ap()
    fc1b_in = nc.dram_tensor("fc1b", [30], f32, kind="ExternalInput").ap()
    fc2wT_in = nc.dram_tensor("fc2wT", [30, 6], f32, kind="ExternalInput").ap()
    fc2b_in = nc.dram_tensor("fc2b", [6], f32, kind="ExternalInput").ap()
    cnnw_in = nc.dram_tensor("cnnw", [6], f32, kind="ExternalInput").ap()
    cnnb_in = nc.dram_tensor("cnnb", [1], f32, kind="ExternalInput").ap()
    corr_in = nc.dram_tensor("corr", [6], f32, kind="ExternalInput").ap()
    out_d = nc.dram_tensor("out_slice", [FM, RPC], f32, kind="ExternalOutput").ap()

    dma_sem = nc.alloc_semaphore("gather_dma")

    with tile.TileContext(nc) as tc, ExitStack() as ctx:
        singles = ctx.enter_context(tc.tile_pool(name="singles", bufs=1))
        pool = ctx.enter_context(tc.tile_pool(name="pool", bufs=3))
        selp = ctx.enter_context(tc.tile_pool(name="selp", bufs=2))
        gpo = ctx.enter_context(tc.tile_pool(name="gpo", bufs=2))
        fpo = ctx.enter_context(tc.tile_pool(name="fpo", bufs=4))
        psA = ctx.enter_context(tc.tile_pool(name="psA", bufs=4, space="PSUM"))
        psB = ctx.enter_context(tc.tile_pool(name="psB", bufs=2, space="PSUM"))
        dram = ctx.enter_context(tc.tile_pool(name="dram", bufs=1, space="DRAM"))

        tables = [dram.tile([NPAD, FM], bf16, name=f"table{i}") for i in range(3)]
        tab_slices = [dram.tile([RPC, FM], bf16, name=f"tab_slice{i}") for i in range(2)]
        fT_sl = [dram.tile([128, RPC], bf16, name=f"fT_sl{i}") for i in range(6)]
        pool6_in = dram.tile([6, 1], f32, name="pool6_in")
        pool6_out = dram.tile([6, 1], f32, name="pool6_out")
        a_scr = dram.tile([6], f32, name="a_scr")

        # ---- constants ----
        ident = singles.tile([128, 128], bf16)
        nc.sync.dma_start(out=ident[:], in_=ident_in[:])
        identf = singles.tile([128, 128], f32)
        nc.vector.tensor_copy(out=identf[:], in_=ident[:])
        W_sb, bb_sb = {}, {}
        for nm in ["f1", "f2", "s1", "s2", "g1", "g2"]:
            W_sb[nm] = singles.tile([FM, FM], bf16, tag=f"W_{nm}", name=f"Wsb_{nm}")
            nc.sync.dma_start(out=W_sb[nm][:], in_=W_in[nm][:])
            bb_sb[nm] = singles.tile([FM, 1], f32, tag=f"bb_{nm}", name=f"bbsb_{nm}")
            nc.sync.dma_start(out=bb_sb[nm][:], in_=b_in[nm].unsqueeze(1))
        pooled_acc = singles.tile([128, 6], f32)
        nc.vector.memset(pooled_acc[:], 0.0)

        idx_sb = {}
        for v in VIEWS:
            C = meta[v][2]
            idx_sb[v] = singles.tile([128, C * 8], i16, tag=f"idx_{v}", name=f"idxsb_{v}")
            nc.sync.dma_start(out=idx_sb[v][:], in_=idx_in[v][:])

        # per-table-buffer state for manual collective/gather dep tracking
        # (custom-DMA APs over DRAM pool tiles are not reliably dep-tracked)
        tabst = [{"ag": None, "preps": []} for _ in range(3)]
        slice_ag = [None, None]   # last AllGather reading tab_slices[i]

        def tab_phase(src_kind, vsrc_l, Wn, ts):
            """tab_slices[ts] = cast_bf16(src @ W) for own rows.
            src 'x': xT input; src 'f': fT_sl[vsrc_l] (both [feat, node])."""
            war = slice_ag[ts]
            for p in range(NPOS):
                cols = slice(p * 128, (p + 1) * 128)
                t_fn = pool.tile([128, 128], bf16, tag="tabin")
                assert src_kind == "f"
                ld = nc.sync.dma_start(out=t_fn[:], in_=fT_sl[vsrc_l][:, cols])
                pm = psB.tile([128, 128], f32, tag="tabps")
                nc.tensor.matmul(pm[:], lhsT=W_sb[Wn][:], rhs=t_fn[:], start=True, stop=True)
                tmid = pool.tile([128, 128], f32, tag="tmid")
                nc.scalar.copy(out=tmid[:], in_=pm[:])
                ptr2 = psB.tile([128, 128], f32, tag="tabps2")
                nc.tensor.transpose(out=ptr2[:], in_=tmid[:], identity=identf[:])
                tb = pool.tile([128, 128], bf16, tag="tbf")
                nc.vector.tensor_copy(out=tb[:], in_=ptr2[:])
                st = nc.sync.dma_start(out=tab_slices[ts][p * 128:(p + 1) * 128, :], in_=tb[:])
                if war is not None:
                    add_dep_helper(st.ins, war.ins, reason="tab_slice WAR")
                yield st

        def allgather_table(tab_stores, ts, buf):
            ag = nc.gpsimd.collective_compute(
                "AllGather", mybir.AluOpType.bypass,
                replica_groups=[list(range(NCORES))],
                ins=[tab_slices[ts][:]], outs=[tables[buf][:]],
            )
            for st in tab_stores:
                add_dep_helper(ag.ins, st.ins, reason="tab_slice RAW")
            for g in tabst[buf]["preps"]:
                add_dep_helper(ag.ins, g.ins, reason="table WAR")
            tabst[buf] = {"ag": ag, "preps": []}
            slice_ag[ts] = ag

        def agg_phase(v, Wn, l_out, buf, from_x=False):
            NLO, NHI, C = meta[v]
            ag = None if from_x else tabst[buf]["ag"]
            sgs = [list(range(s, min(s + SG, NPOS))) for s in range(0, NPOS, SG)]
            chunk_base = 0  # global chunk counter
            idx_col = 0     # column offset into idx_sb (units of 16 idxs)
            for sg in sgs:
                nlo = int(sum(NLO[p] for p in sg))
                nhi = int(sum(NHI[p] for p in sg))
                nch_sg = nlo + nhi
                # stream this supergroup's selector chunks (contiguous)
                selsb = selp.tile([128, max(nch_sg, 1) * 128], bf16, tag="sel")
                nc.sync.dma_start(
                    out=selsb[:],
                    in_=sel_in[v][:, chunk_base * 128:(chunk_base + max(nch_sg, 1)) * 128])
                glo = gpo.tile([128, max(nlo, 1), 128], bf16, tag="glo")
                ghi = gpo.tile([128, max(nhi, 1), 128], bf16, tag="ghi")
                GMAXC = int(os.environ.get("KERNEL_GMAXC", "8"))  # chunks per dma_gather (1024 idxs default; >=2048 hangs SWDGE)
                for half_i, (nh, gt, lohi) in enumerate(
                        (((nlo, glo, (0, HALF)), (nhi, ghi, (HALF, NPAD))))):
                    for g0 in range(0, nh, GMAXC):
                        gn = min(GMAXC, nh - g0)
                        src_tab = (xfull_in if from_x else tables[buf])
                        gi = nc.gpsimd.dma_gather(
                            out_ap=gt[:, g0:g0 + gn, :],
                            in_ap=src_tab[lohi[0]:lohi[1], :],
                            idxs_ap=idx_sb[v][:, idx_col:idx_col + gn * 8],
                            num_idxs=gn * 128, num_idxs_reg=gn * 128, elem_size=128,
                        )
                        if not from_x:
                            add_dep_helper(gi.ins, ag.ins, reason="table RAW")
                            tabst[buf]["preps"].append(gi)
                        idx_col += gn * 8
                # chunk order in sel array: [lo(p0)..lo(pk)] then [hi(p0)..hi(pk)]
                lo_off, off = {}, 0
                for p in sg:
                    lo_off[p] = off
                    off += int(NLO[p])
                hi_off, off = {}, 0
                for p in sg:
                    hi_off[p] = off
                    off += int(NHI[p])
                for p in sg:
                    ps = psA.tile([128, 128], f32, tag="agg")
                    nch = int(NLO[p] + NHI[p])
                    ci = 0
                    for k in range(int(NLO[p])):
                        cg = lo_off[p] + k  # sel col block within supergroup
                        nc.tensor.matmul(ps[:], lhsT=glo[:, lo_off[p] + k, :],
                                         rhs=selsb[:, cg * 128:(cg + 1) * 128],
                                         start=(ci == 0), stop=(ci == nch - 1))
                        ci += 1
                    for k in range(int(NHI[p])):
                        cg = nlo + hi_off[p] + k
                        nc.tensor.matmul(ps[:], lhsT=ghi[:, hi_off[p] + k, :],
                                         rhs=selsb[:, cg * 128:(cg + 1) * 128],
                                         start=(ci == 0), stop=(ci == nch - 1))
                        ci += 1
                    if from_x:
                        # aggregate-then-transform: psum holds (A@x)^T [fi,dst];
                        # apply W on-PE before bias+relu.
                        aggT = fpo.tile([128, 128], bf16, tag="aggT")
                        nc.scalar.copy(out=aggT[:], in_=ps[:])
                        ps2 = psB.tile([128, 128], f32, tag="tabps")
                        nc.tensor.matmul(ps2[:], lhsT=W_sb[Wn][:], rhs=aggT[:],
                                         start=True, stop=True)
                        ps = ps2
                    # postprocess: fT = relu(agg + b), fused row-sum for pooling
                    ft = fpo.tile([128, 128], bf16, tag="ftile")
                    racc = fpo.tile([128, 1], f32, tag="racc")
                    nc.scalar.activation(
                        out=ft[:], in_=ps[:], func=mybir.ActivationFunctionType.Relu,
                        bias=bb_sb[Wn][:, 0:1], accum_out=racc[:])
                    nc.vector.tensor_tensor(out=pooled_acc[:, l_out:l_out + 1],
                                            in0=pooled_acc[:, l_out:l_out + 1],
                                            in1=racc[:], op=mybir.AluOpType.add)
                    nc.scalar.dma_start(out=fT_sl[l_out][:, p * 128:(p + 1) * 128],
                                        in_=ft[:])
                chunk_base += nch_sg

        scope = os.environ.get("KERNEL_SCOPE", "full")
        if scope == "full":
            # schedule: tab f1, AG f1, tab s1, AG s1, agg f1, tab g1, AG g1,
            # agg s1, tab f2, AG f2, agg g1, ... so each AllGather overlaps
            # the previous layer's aggregation (ping-pong table buffers).
            plans = []
            for i, (v, ln) in enumerate(LAYERS):
                nm = f"{v}{ln}"
                src = ("x", None) if ln == 1 else ("f", 2 * VIEWS.index(v))
                plans.append({"v": v, "nm": nm, "src": src, "l_out": 2 * VIEWS.index(v) + ln - 1,
                              "buf": VIEWS.index(v), "ts": i % 2})

            def do_tab(i):
                p = plans[i]
                stores = list(tab_phase(p["src"][0], p["src"][1], p["nm"], p["ts"]))
                allgather_table(stores, p["ts"], p["buf"])

            def do_agg(i, from_x=False):
                p = plans[i]
                agg_phase(p["v"], p["nm"], p["l_out"], p["buf"], from_x=from_x)

            # layer-1 aggregations gather raw x rows from the replicated input
            # table (no tab phase, no AllGather) and post-multiply by W; only
            # the second layers need table AllGathers.
            do_agg(0, from_x=True)
            do_tab(3)
            do_agg(1, from_x=True)
            do_tab(4)
            do_agg(2, from_x=True)
            do_tab(5)
            do_agg(3)
            do_agg(4)
            do_agg(5)
        else:
            plans = [{"src": ("x", None), "nm": "f1", "ts": 0, "buf": 0, "v": "f",
                      "l_out": 0}]
            if scope in ("agg1", "f1out"):
                agg_phase("f", "f1", 0, 0, from_x=True)

        # ---- pooled -> SE attention scalars ----
        pool_red = singles.tile([128, 6], f32)
        nc.gpsimd.partition_all_reduce(pool_red[:], pooled_acc[:], 128,
                                       bass_isa.ReduceOp.add)
        nc.sync.dma_start(out=pool6_in[:], in_=pool_red[0:1, 0:6])
        nc.gpsimd.collective_compute(
            "AllReduce", mybir.AluOpType.add,
            replica_groups=[list(range(NCORES))],
            ins=[pool6_in[:]], outs=[pool6_out[:]],
        )
        pvec2 = singles.tile([6, 1], f32)
        nc.sync.dma_start(out=pvec2[:], in_=pool6_out[:])
        corr_sb = singles.tile([6, 1], f32)
        nc.sync.dma_start(out=corr_sb[:], in_=corr_in.unsqueeze(1))
        # remove pad-column relu(bias) pollution, then mean
        nc.vector.tensor_tensor(out=pvec2[:], in0=pvec2[:], in1=corr_sb[:],
                                op=mybir.AluOpType.subtract)
        nc.vector.tensor_scalar_mul(pvec2[:], pvec2[:], 1.0 / (N * FM))
        fc1wT = singles.tile([6, 30], f32)
        nc.sync.dma_start(out=fc1wT[:], in_=fc1wT_in[:])
        fc1b = singles.tile([30, 1], f32)
        nc.sync.dma_start(out=fc1b[:], in_=fc1b_in.unsqueeze(1))
        fc2wT = singles.tile([30, 6], f32)
        nc.sync.dma_start(out=fc2wT[:], in_=fc2wT_in[:])
        fc2b = singles.tile([6, 1], f32)
        nc.sync.dma_start(out=fc2b[:], in_=fc2b_in.unsqueeze(1))
        pz1 = psB.tile([30, 1], f32, tag="tabps")
        nc.tensor.matmul(pz1[:], lhsT=fc1wT[:], rhs=pvec2[:], start=True, stop=True)
        z1 = singles.tile([30, 1], f32)
        nc.vector.tensor_tensor(out=z1[:], in0=pz1[:], in1=fc1b[:], op=mybir.AluOpType.add)
        nc.vector.tensor_scalar_max(z1[:], z1[:], 0.0)
        pz2 = psB.tile([6, 1], f32, tag="tabps")
        nc.tensor.matmul(pz2[:], lhsT=fc2wT[:], rhs=z1[:], start=True, stop=True)
        z2 = singles.tile([6, 1], f32)
        nc.vector.tensor_tensor(out=z2[:], in0=pz2[:], in1=fc2b[:], op=mybir.AluOpType.add)
        av = singles.tile([6, 1], f32)
        nc.scalar.activation(out=av[:], in_=z2[:], func=mybir.ActivationFunctionType.Sigmoid)
        nc.sync.dma_start(out=a_scr[:], in_=av[:, 0])
        a_b = singles.tile([128, 6], f32)
        nc.gpsimd.dma_start(out=a_b[:], in_=a_scr[:].partition_broadcast(128))
        cnnw_b = singles.tile([128, 6], f32)
        nc.gpsimd.dma_start(out=cnnw_b[:], in_=cnnw_in.partition_broadcast(128))
        cnnb_b = singles.tile([128, 1], f32)
        nc.gpsimd.dma_start(out=cnnb_b[:], in_=cnnb_in.partition_broadcast(128))

        # ---- final combine: outT = sum_l cnnw_l * relu(a_l * fT_l) + cnn_b ----
        if scope == "f1out":
            for p in range(NPOS):
                cols = slice(p * 128, (p + 1) * 128)
                fl0 = fpo.tile([128, 128], bf16, tag="fin", name=f"fl0_{p}")
                nc.sync.dma_start(out=fl0[:], in_=fT_sl[0][:, cols])
                fo = fpo.tile([128, 128], f32, tag="ftmp", name=f"fo_{p}")
                nc.vector.tensor_copy(out=fo[:], in_=fl0[:])
                nc.sync.dma_start(out=out_d[:, cols], in_=fo[:])
        nlayers = 6 if scope == "full" else 1
        for p in range(NPOS) if scope != "f1out" else []:
            cols = slice(p * 128, (p + 1) * 128)
            acc = fpo.tile([128, 128], f32, tag="facc")
            for l in range(nlayers):
                fl = fpo.tile([128, 128], bf16, tag="fin")
                nc.sync.dma_start(out=fl[:], in_=fT_sl[l][:, cols])
                t = fpo.tile([128, 128], f32, tag="ftmp")
                nc.scalar.activation(out=t[:], in_=fl[:],
                                     func=mybir.ActivationFunctionType.Relu,
                                     scale=a_b[:, l:l + 1])
                if l == 0:
                    nc.vector.tensor_scalar_mul(acc[:], t[:], cnnw_b[:, 0:1])
                else:
                    nc.vector.tensor_scalar_mul(t[:], t[:], cnnw_b[:, l:l + 1])
                    nc.vector.tensor_tensor(out=acc[:], in0=acc[:], in1=t[:],
                                            op=mybir.AluOpType.add)
            nc.vector.tensor_scalar_add(acc[:], acc[:], cnnb_b[:, 0:1])
            nc.sync.dma_start(out=out_d[:, cols], in_=acc[:])

    nc.compile()
    _split_multiwaits(nc)
    return nc


def kernel(**inputs):
    global _last_exec_time_ns
    inputs = {k: np.asarray(v) for k, v in inputs.items()}

    meta = {}
    perview = {}
    for v in VIEWS:
        idx_arrs, sel_arrs, NLO, NHI = _prep_view(
            inputs[f"edges_{v}"].astype(np.int64), inputs[f"ew_{v}"])
        meta[v] = (NLO, NHI, int((NLO + NHI).sum()))
        perview[v] = (idx_arrs, sel_arrs)

    nc = _build(meta)

    x_pad = np.zeros((NPAD, FM), np.float32)
    x_pad[:N, :] = inputs["x_m"].astype(np.float32)
    x_pad = np.ascontiguousarray(x_pad).astype(bfnp)
    ident_np = np.eye(128, dtype=bfnp)
    # pad dst columns (node ids >= N, all on core 7) read relu(bias) into the
    # pooled sum; precompute the exact pollution per layer and subtract it.
    npad_cols = NPAD - N
    corr = np.array(
        [npad_cols * np.maximum(inputs[f"b_{nm}"].astype(np.float64), 0).sum()
         for nm in ["f1", "f2", "s1", "s2", "g1", "g2"]], np.float32)

    in_maps = []
    for c in range(NCORES):
        m = {
            "x_full": x_pad,
            "ident": ident_np,
            "fc1wT": inputs["fc1_w"].T.astype(np.float32).copy(),
            "fc1b": inputs["fc1_b"].astype(np.float32),
            "fc2wT": inputs["fc2_w"].T.astype(np.float32).copy(),
            "fc2b": inputs["fc2_b"].astype(np.float32),
            "cnnw": inputs["cnn_w"].astype(np.float32),
            "cnnb": inputs["cnn_b"].astype(np.float32),
            "corr": corr,
        }
        for nm in ["f1", "f2", "s1", "s2", "g1", "g2"]:
            m[f"W_{nm}"] = inputs[f"W_{nm}"].astype(bfnp)
            m[f"b_{nm}"] = inputs[f"b_{nm}"].astype(np.float32)
        for v in VIEWS:
            idx_arrs, sel_arrs = perview[v]
            m[f"idx_{v}"] = idx_arrs[c]
            m[f"sel_{v}"] = sel_arrs[c]
        in_maps.append(m)

    trace = os.environ.get("KERNEL_TRACE", "0") == "1"
    kw = {}
    if trace:
        td = os.environ.get("KERNEL_TRACE_DIR")
        if td:
            os.makedirs(td, exist_ok=True)
            kw["tmpdir"] = td
    res = run_bass_kernel_spmd(nc, in_maps, list(range(NCORES)), trace=trace, **kw)
    _last_exec_time_ns = res.exec_time_ns
    outT = np.concatenate([res.results[c]["out_slice"] for c in range(NCORES)], axis=1)
    return np.ascontiguousarray(outT.T[:N]).astype(np.float32)



# revision 15
# speedup vs baseline: 1.4252x; 1.0005x over previous
"""GCN message-passing kernel for 8 Trainium2 NeuronCores.

Strategy: shard destination nodes across cores (6272 rows/core). Each core
aggregates all edges targeting its rows by gathering source rows with
synchronous SWDGE dma_gather (1024 idxs/fire; >1024 hangs this ucode, and
every fire blocks the Pool engine ~8.5us regardless of mode, so sync issue
is optimal) and contracting each 128-edge chunk against a host-precomputed
one-hot selector (streamed from HBM) on the PE array. Layer-1 aggregations
use aggregate-then-transform (A@(xW) = (A@x)W): they gather raw x rows from
a replicated host-uploaded table and apply W on-PE afterwards, so no tab
phase or AllGather precedes them and the gather pipeline starts at t=0;
only the three layer-2 tables are computed+AllGathered (into 3 dedicated
DRAM buffers), overlapping the preceding aggregations. The aggregation runs
transposed (psum[feat, dst]) so bias+relu+row-sum fuse into one
Activation-engine op. SE attention + 1x1 conv are tiny and replicated; the
final output is produced transposed and fixed up on host.
"""
import os
import sys

sys.path.insert(0, "/opt/trn_rl_repo")

from contextlib import ExitStack

import ml_dtypes
import numpy as np

import concourse.bacc as bacc
import concourse.tile as tile
from concourse.tile import add_dep_helper
from concourse import bass_isa, mybir
from concourse.bass_utils import run_bass_kernel_spmd

N = 50000
FM = 128
E = 800000
NCORES = 8
NPOS = 49                  # 128-row tiles per core
RPC = NPOS * 128           # 6272 rows per core
NPAD = NCORES * RPC        # 50176
HALF = NPAD // 2           # 25088 (int16 gather index limit per table half)
SG = 4                     # positions per gather supergroup
VIEWS = ("f", "s", "g")
LAYERS = [("f", 1), ("s", 1), ("g", 1), ("f", 2), ("s", 2), ("g", 2)]

f32 = mybir.dt.float32
bf16 = mybir.dt.bfloat16
i16 = mybir.dt.int16
bfnp = ml_dtypes.bfloat16

_last_exec_time_ns = None


def _split_multiwaits(nc):
    """This walrus build accepts only ONE sync-wait per instruction; split
    extras into preceding same-engine single-wait NoOps (sequencer executes
    waits in program order, so semantics are preserved)."""
    n = 0
    for fn in nc.m.functions:
        for bb in fn.blocks:
            newlist = []
            for inst in bb.instructions:
                si = inst.sync_info
                if si is not None and len(si.on_wait) > 1:
                    waits = list(si.on_wait)
                    for w in waits[:-1]:
                        nop = mybir.InstNoOp(name=f"WSPL-{nc.next_id()}", ins=[], outs=[])
                        nop.engine = inst.engine
                        nop.sync_info = mybir.SyncInfo(on_wait=[w], on_update=[])
                        newlist.append(nop)
                        n += 1
                    si.on_wait = [waits[-1]]
                newlist.append(inst)
            bb.instructions = newlist
    return n


def _prep_view(edges, ew):
    """Host edge preprocessing for one view: append self-loops, compute the
    symmetric GCN normalization, shard by destination across cores, group by
    (dst tile, src half), pad runs to 128-edge chunks (uniform across cores).

    Returns (idx_arrs, sel_arrs, NLO, NHI): per-core SWDGE index arrays and
    precomputed one-hot selector chunks ([128 edge-slot partitions, C*128
    dst columns], bf16, selector value = the edge's GCN norm weight)."""
    src = np.concatenate([edges[0], np.arange(N, dtype=np.int64)])
    dst = np.concatenate([edges[1], np.arange(N, dtype=np.int64)])
    w = np.concatenate([ew.astype(np.float64), np.ones(N)])
    deg = np.bincount(dst, weights=w, minlength=N)
    dis = 1.0 / np.sqrt(deg)
    norm = (dis[src] * w * dis[dst]).astype(np.float32)

    core = dst // RPC
    pos = (dst % RPC) // 128
    dstrel = (dst % 128).astype(np.int64)
    half = (src >= HALF).astype(np.int64)
    idx = (src - HALF * half).astype(np.int16)

    # counts[c, p, h]
    key = (core * NPOS + pos) * 2 + half
    counts = np.bincount(key, minlength=NCORES * NPOS * 2).reshape(NCORES, NPOS, 2)
    chunks = -(-counts // 128)  # ceil
    NLO = chunks[:, :, 0].max(axis=0)
    NHI = chunks[:, :, 1].max(axis=0)

    order = np.lexsort((half, pos, core))
    norm_s, dstrel_s, idx_s, key_s = (
        norm[order], dstrel[order], idx[order], key[order])
    starts = np.searchsorted(key_s, np.arange(NCORES * NPOS * 2))
    ends = np.searchsorted(key_s, np.arange(NCORES * NPOS * 2), side="right")

    C = int((NLO + NHI).sum())
    idx_arrs, sel_arrs = [], []
    sgs = [list(range(s, min(s + SG, NPOS))) for s in range(0, NPOS, SG)]
    for c in range(NCORES):
        idx_a = np.zeros(C * 128, np.int16)
        dr_a = np.zeros(C * 128, np.int64)
        w_a = np.zeros(C * 128, np.float32)
        off = 0
        for sg in sgs:
            for h in range(2):
                for p in sg:
                    k = (c * NPOS + p) * 2 + h
                    s0, e0 = starts[k], ends[k]
                    n = e0 - s0
                    nch = (NLO if h == 0 else NHI)[p]
                    idx_a[off:off + n] = idx_s[s0:e0]
                    dr_a[off:off + n] = dstrel_s[s0:e0]
                    w_a[off:off + n] = norm_s[s0:e0]
                    off += nch * 128
        assert off == C * 128
        # device layouts
        idx_wrapped = np.tile(idx_a.reshape(-1, 16).T, (8, 1)).copy()  # [128, C*8]
        sel_flat = np.zeros((C * 128, 128), np.float32)
        sel_flat[np.arange(C * 128), dr_a] = w_a
        sel_dev = np.ascontiguousarray(
            sel_flat.reshape(C, 128, 128).transpose(1, 0, 2).reshape(128, C * 128)
        ).astype(bfnp)
        idx_arrs.append(idx_wrapped)
        sel_arrs.append(sel_dev)
    return idx_arrs, sel_arrs, NLO.astype(int), NHI.astype(int)


def _build(meta):
    """Build the SPMD program. meta[v] = (NLO, NHI, C) per view."""
    nc = bacc.Bacc("TRN2", target_bir_lowering=False, debug=False,
                   num_devices=NCORES)

    # ---- I/O ----
    xfull_in = nc.dram_tensor("x_full", [NPAD, FM], bf16, kind="ExternalInput").ap()
    W_in, b_in = {}, {}
    for nm in ["f1", "f2", "s1", "s2", "g1", "g2"]:
        W_in[nm] = nc.dram_tensor(f"W_{nm}", [FM, FM], bf16, kind="ExternalInput").ap()
        b_in[nm] = nc.dram_tensor(f"b_{nm}", [FM], f32, kind="ExternalInput").ap()
    idx_in, sel_in = {}, {}
    for v in VIEWS:
        C = meta[v][2]
        idx_in[v] = nc.dram_tensor(f"idx_{v}", [128, C * 8], i16, kind="ExternalInput").ap()
        sel_in[v] = nc.dram_tensor(f"sel_{v}", [128, C * 128], bf16, kind="ExternalInput").ap()
    ident_in = nc.dram_tensor("ident", [128, 128], bf16, kind="ExternalInput").ap()
    fc1wT_in = nc.dram_tensor("fc1wT", [6, 30], f32, kind="ExternalInput").ap()
    fc1b_in = nc.dram_tensor("fc1b", [30], f32, kind="ExternalInput").ap()
    fc2wT_in = nc.dram_tensor("fc2wT", [30, 6], f32, kind="ExternalInput").ap()
    fc2b_in = nc.dram_tensor("fc2b", [6], f32, kind="ExternalInput").ap()
    cnnw_in = nc.dram_tensor("cnnw", [6], f32, kind="ExternalInput").ap()
    cnnb_in = nc.dram_tensor("cnnb", [1], f32, kind="ExternalInput").ap()
    corr_in = nc.dram_tensor("corr", [6], f32, kind="ExternalInput").ap()
    out_d = nc.dram_tensor("out_slice", [FM, RPC], f32, kind="ExternalOutput").ap()

    dma_sem = nc.alloc_semaphore("gather_dma")

    with tile.TileContext(nc) as tc, ExitStack() as ctx:
        singles = ctx.enter_context(tc.tile_pool(name="singles", bufs=1))
        pool = ctx.enter_context(tc.tile_pool(name="pool", bufs=3))
        selp = ctx.enter_context(tc.tile_pool(name="selp", bufs=2))
        gpo = ctx.enter_context(tc.tile_pool(name="gpo", bufs=2))
        fpo = ctx.enter_context(tc.tile_pool(name="fpo", bufs=4))
        psA = ctx.enter_context(tc.tile_pool(name="psA", bufs=4, space="PSUM"))
        psB = ctx.enter_context(tc.tile_pool(name="psB", bufs=2, space="PSUM"))
        dram = ctx.enter_context(tc.tile_pool(name="dram", bufs=1, space="DRAM"))

        tables = [dram.tile([NPAD, FM], bf16, name=f"table{i}") for i in range(3)]
        tab_slices = [dram.tile([RPC, FM], bf16, name=f"tab_slice{i}") for i in range(2)]
        fT_sl = [dram.tile([128, RPC], bf16, name=f"fT_sl{i}") for i in range(6)]
        pool6_in = dram.tile([6, 1], f32, name="pool6_in")
        pool6_out = dram.tile([6, 1], f32, name="pool6_out")
        a_scr = dram.tile([6], f32, name="a_scr")

        # ---- constants ----
        ident = singles.tile([128, 128], bf16)
        nc.sync.dma_start(out=ident[:], in_=ident_in[:])
        identf = singles.tile([128, 128], f32)
        nc.vector.tensor_copy(out=identf[:], in_=ident[:])
        W_sb, bb_sb = {}, {}
        for nm in ["f1", "f2", "s1", "s2", "g1", "g2"]:
            W_sb[nm] = singles.tile([FM, FM], bf16, tag=f"W_{nm}", name=f"Wsb_{nm}")
            nc.sync.dma_start(out=W_sb[nm][:], in_=W_in[nm][:])
            bb_sb[nm] = singles.tile([FM, 1], f32, tag=f"bb_{nm}", name=f"bbsb_{nm}")
            nc.sync.dma_start(out=bb_sb[nm][:], in_=b_in[nm].unsqueeze(1))
        pooled_acc = singles.tile([128, 6], f32)
        nc.vector.memset(pooled_acc[:], 0.0)

        idx_sb = {}
        for v in VIEWS:
            C = meta[v][2]
            idx_sb[v] = singles.tile([128, C * 8], i16, tag=f"idx_{v}", name=f"idxsb_{v}")
            nc.sync.dma_start(out=idx_sb[v][:], in_=idx_in[v][:])

        # per-table-buffer state for manual collective/gather dep tracking
        # (custom-DMA APs over DRAM pool tiles are not reliably dep-tracked)
        tabst = [{"ag": None, "preps": []} for _ in range(3)]
        slice_ag = [None, None]   # last AllGather reading tab_slices[i]

        def tab_phase(src_kind, vsrc_l, Wn, ts):
            """tab_slices[ts] = cast_bf16(src @ W) for own rows.
            src 'x': xT input; src 'f': fT_sl[vsrc_l] (both [feat, node])."""
            war = slice_ag[ts]
            for p in range(NPOS):
                cols = slice(p * 128, (p + 1) * 128)
                t_fn = pool.tile([128, 128], bf16, tag="tabin")
                assert src_kind == "f"
                ld = nc.sync.dma_start(out=t_fn[:], in_=fT_sl[vsrc_l][:, cols])
                pm = psB.tile([128, 128], f32, tag="tabps")
                nc.tensor.matmul(pm[:], lhsT=W_sb[Wn][:], rhs=t_fn[:], start=True, stop=True)
                tmid = pool.tile([128, 128], f32, tag="tmid")
                nc.scalar.copy(out=tmid[:], in_=pm[:])
                ptr2 = psB.tile([128, 128], f32, tag="tabps2")
                nc.tensor.transpose(out=ptr2[:], in_=tmid[:], identity=identf[:])
                tb = pool.tile([128, 128], bf16, tag="tbf")
                nc.vector.tensor_copy(out=tb[:], in_=ptr2[:])
                st = nc.sync.dma_start(out=tab_slices[ts][p * 128:(p + 1) * 128, :], in_=tb[:])
                if war is not None:
                    add_dep_helper(st.ins, war.ins, reason="tab_slice WAR")
                yield st

        def allgather_table(tab_stores, ts, buf):
            ag = nc.gpsimd.collective_compute(
                "AllGather", mybir.AluOpType.bypass,
                replica_groups=[list(range(NCORES))],
                ins=[tab_slices[ts][:]], outs=[tables[buf][:]],
            )
            for st in tab_stores:
                add_dep_helper(ag.ins, st.ins, reason="tab_slice RAW")
            for g in tabst[buf]["preps"]:
                add_dep_helper(ag.ins, g.ins, reason="table WAR")
            tabst[buf] = {"ag": ag, "preps": []}
            slice_ag[ts] = ag

        def agg_phase(v, Wn, l_out, buf, from_x=False):
            NLO, NHI, C = meta[v]
            ag = None if from_x else tabst[buf]["ag"]
            sgs = [list(range(s, min(s + SG, NPOS))) for s in range(0, NPOS, SG)]
            chunk_base = 0  # global chunk counter
            idx_col = 0     # column offset into idx_sb (units of 16 idxs)
            for sg in sgs:
                nlo = int(sum(NLO[p] for p in sg))
                nhi = int(sum(NHI[p] for p in sg))
                nch_sg = nlo + nhi
                # stream this supergroup's selector chunks (contiguous)
                selsb = selp.tile([128, max(nch_sg, 1) * 128], bf16, tag="sel")
                nc.sync.dma_start(
                    out=selsb[:],
                    in_=sel_in[v][:, chunk_base * 128:(chunk_base + max(nch_sg, 1)) * 128])
                glo = gpo.tile([128, max(nlo, 1), 128], bf16, tag="glo")
                ghi = gpo.tile([128, max(nhi, 1), 128], bf16, tag="ghi")
                GMAXC = int(os.environ.get("KERNEL_GMAXC", "8"))  # chunks per dma_gather (1024 idxs default; >=2048 hangs SWDGE)
                for half_i, (nh, gt, lohi) in enumerate(
                        (((nlo, glo, (0, HALF)), (nhi, ghi, (HALF, NPAD))))):
                    for g0 in range(0, nh, GMAXC):
                        gn = min(GMAXC, nh - g0)
                        src_tab = (xfull_in if from_x else tables[buf])
                        gi = nc.gpsimd.dma_gather(
                            out_ap=gt[:, g0:g0 + gn, :],
                            in_ap=src_tab[lohi[0]:lohi[1], :],
                            idxs_ap=idx_sb[v][:, idx_col:idx_col + gn * 8],
                            num_idxs=gn * 128, num_idxs_reg=gn * 128, elem_size=128,
                        )
                        if not from_x:
                            add_dep_helper(gi.ins, ag.ins, reason="table RAW")
                            tabst[buf]["preps"].append(gi)
                        idx_col += gn * 8
                # chunk order in sel array: [lo(p0)..lo(pk)] then [hi(p0)..hi(pk)]
                lo_off, off = {}, 0
                for p in sg:
                    lo_off[p] = off
                    off += int(NLO[p])
                hi_off, off = {}, 0
                for p in sg:
                    hi_off[p] = off
                    off += int(NHI[p])
                for p in sg:
                    ps = psA.tile([128, 128], f32, tag="agg")
                    nch = int(NLO[p] + NHI[p])
                    ci = 0
                    for k in range(int(NLO[p])):
                        cg = lo_off[p] + k  # sel col block within supergroup
                        nc.tensor.matmul(ps[:], lhsT=glo[:, lo_off[p] + k, :],
                                         rhs=selsb[:, cg * 128:(cg + 1) * 128],
                                         start=(ci == 0), stop=(ci == nch - 1))
                        ci += 1
                    for k in range(int(NHI[p])):
                        cg = nlo + hi_off[p] + k
                        nc.tensor.matmul(ps[:], lhsT=ghi[:, hi_off[p] + k, :],
                                         rhs=selsb[:, cg * 128:(cg + 1) * 128],
                                         start=(ci == 0), stop=(ci == nch - 1))
                        ci += 1
                    if from_x:
                        # aggregate-then-transform: psum holds (A@x)^T [fi,dst];
                        # apply W on-PE before bias+relu.
                        aggT = fpo.tile([128, 128], bf16, tag="aggT")
                        nc.scalar.copy(out=aggT[:], in_=ps[:])
                        ps2 = psB.tile([128, 128], f32, tag="tabps")
                        nc.tensor.matmul(ps2[:], lhsT=W_sb[Wn][:], rhs=aggT[:],
                                         start=True, stop=True)
                        ps = ps2
                    # postprocess: fT = relu(agg + b), fused row-sum for pooling
                    ft = fpo.tile([128, 128], bf16, tag="ftile")
                    racc = fpo.tile([128, 1], f32, tag="racc")
                    nc.scalar.activation(
                        out=ft[:], in_=ps[:], func=mybir.ActivationFunctionType.Relu,
                        bias=bb_sb[Wn][:, 0:1], accum_out=racc[:])
                    nc.vector.tensor_tensor(out=pooled_acc[:, l_out:l_out + 1],
                                            in0=pooled_acc[:, l_out:l_out + 1],
                                            in1=racc[:], op=mybir.AluOpType.add)
                    nc.scalar.dma_start(out=fT_sl[l_out][:, p * 128:(p + 1) * 128],
                                        in_=ft[:])
                chunk_base += nch_sg

        scope = os.environ.get("KERNEL_SCOPE", "full")
        if scope == "full":
            # schedule: tab f1, AG f1, tab s1, AG s1, agg f1, tab g1, AG g1,
            # agg s1, tab f2, AG f2, agg g1, ... so each AllGather overlaps
            # the previous layer's aggregation (ping-pong table buffers).
            plans = []
            for i, (v, ln) in enumerate(LAYERS):
                nm = f"{v}{ln}"
                src = ("x", None) if ln == 1 else ("f", 2 * VIEWS.index(v))
                plans.append({"v": v, "nm": nm, "src": src, "l_out": 2 * VIEWS.index(v) + ln - 1,
                              "buf": VIEWS.index(v), "ts": i % 2})

            def do_tab(i):
                p = plans[i]
                stores = list(tab_phase(p["src"][0], p["src"][1], p["nm"], p["ts"]))
                allgather_table(stores, p["ts"], p["buf"])

            def do_agg(i, from_x=False):
                p = plans[i]
                agg_phase(p["v"], p["nm"], p["l_out"], p["buf"], from_x=from_x)

            # layer-1 aggregations gather raw x rows from the replicated input
            # table (no tab phase, no AllGather) and post-multiply by W; only
            # the second layers need table AllGathers.
            do_agg(0, from_x=True)
            do_tab(3)
            do_agg(1, from_x=True)
            do_tab(4)
            do_agg(2, from_x=True)
            do_tab(5)
            do_agg(3)
            do_agg(4)
            do_agg(5)
        else:
            plans = [{"src": ("x", None), "nm": "f1", "ts": 0, "buf": 0, "v": "f",
                      "l_out": 0}]
            if scope in ("agg1", "f1out"):
                agg_phase("f", "f1", 0, 0, from_x=True)

        # ---- pooled -> SE attention scalars ----
        pool_red = singles.tile([128, 6], f32)
        nc.gpsimd.partition_all_reduce(pool_red[:], pooled_acc[:], 128,
                                       bass_isa.ReduceOp.add)
        nc.sync.dma_start(out=pool6_in[:], in_=pool_red[0:1, 0:6])
        nc.gpsimd.collective_compute(
            "AllReduce", mybir.AluOpType.add,
            replica_groups=[list(range(NCORES))],
            ins=[pool6_in[:]], outs=[pool6_out[:]],
        )
        pvec2 = singles.tile([6, 1], f32)
        nc.sync.dma_start(out=pvec2[:], in_=pool6_out[:])
        corr_sb = singles.tile([6, 1], f32)
        nc.sync.dma_start(out=corr_sb[:], in_=corr_in.unsqueeze(1))
        # remove pad-column relu(bias) pollution, then mean
        nc.vector.tensor_tensor(out=pvec2[:], in0=pvec2[:], in1=corr_sb[:],
                                op=mybir.AluOpType.subtract)
        nc.vector.tensor_scalar_mul(pvec2[:], pvec2[:], 1.0 / (N * FM))
        fc1wT = singles.tile([6, 30], f32)
        nc.sync.dma_start(out=fc1wT[:], in_=fc1wT_in[:])
        fc1b = singles.tile([30, 1], f32)
        nc.sync.dma_start(out=fc1b[:], in_=fc1b_in.unsqueeze(1))
        fc2wT = singles.tile([30, 6], f32)
        nc.sync.dma_start(out=fc2wT[:], in_=fc2wT_in[:])
        fc2b = singles.tile([6, 1], f32)
        nc.sync.dma_start(out=fc2b[:], in_=fc2b_in.unsqueeze(1))
        pz1 = psB.tile([30, 1], f32, tag="tabps")
        nc.tensor.matmul(pz1[:], lhsT=fc1wT[:], rhs=pvec2[:], start=True, stop=True)
        z1 = singles.tile([30, 1], f32)
        nc.vector.tensor_tensor(out=z1[:], in0=pz1[:], in1=fc1b[:], op=mybir.AluOpType.add)
        nc.vector.tensor_scalar_max(z1[:], z1[:], 0.0)
        pz2 = psB.tile([6, 1], f32, tag="tabps")
        nc.tensor.matmul(pz2[:], lhsT=fc2wT[:], rhs=z1[:], start=True, stop=True)
        z2 = singles.tile([6, 1], f32)
        nc.vector.tensor_tensor(out=z2[:], in0=pz2[:], in1=fc2b[:], op=mybir.AluOpType.add)
        av = singles.tile([6, 1], f32)
        nc.scalar.activation(out=av[:], in_=z2[:], func=mybir.ActivationFunctionType.Sigmoid)
        cnnw6 = singles.tile([6, 1], f32)
        nc.sync.dma_start(out=cnnw6[:], in_=cnnw_in.unsqueeze(1))
        # a >= 0 (sigmoid) and fT >= 0 (post-relu), so relu(a*fT) == a*fT and
        # the combine is linear: out = sum_l (a_l*cnnw_l)*fT_l + cnnb.
        wv = singles.tile([6, 1], f32)
        nc.vector.tensor_tensor(out=wv[:], in0=av[:], in1=cnnw6[:],
                                op=mybir.AluOpType.mult)
        nc.sync.dma_start(out=a_scr[:], in_=wv[:, 0])
        w_b = singles.tile([128, 6], f32)
        nc.gpsimd.dma_start(out=w_b[:], in_=a_scr[:].partition_broadcast(128))
        cnnb_b = singles.tile([128, 1], f32)
        nc.gpsimd.dma_start(out=cnnb_b[:], in_=cnnb_in.partition_broadcast(128))

        # ---- final combine: outT = sum_l cnnw_l * relu(a_l * fT_l) + cnn_b ----
        if scope == "f1out":
            for p in range(NPOS):
                cols = slice(p * 128, (p + 1) * 128)
                fl0 = fpo.tile([128, 128], bf16, tag="fin", name=f"fl0_{p}")
                nc.sync.dma_start(out=fl0[:], in_=fT_sl[0][:, cols])
                fo = fpo.tile([128, 128], f32, tag="ftmp", name=f"fo_{p}")
                nc.vector.tensor_copy(out=fo[:], in_=fl0[:])
                nc.sync.dma_start(out=out_d[:, cols], in_=fo[:])
        nlayers = 6 if scope == "full" else 1
        for p in range(NPOS) if scope != "f1out" else []:
            cols = slice(p * 128, (p + 1) * 128)
            acc = fpo.tile([128, 128], f32, tag="facc")
            for l in range(nlayers):
                fl = fpo.tile([128, 128], bf16, tag="fin")
                nc.sync.dma_start(out=fl[:], in_=fT_sl[l][:, cols])
                if l == 0:
                    nc.vector.tensor_scalar_mul(acc[:], fl[:], w_b[:, 0:1])
                else:
                    t = fpo.tile([128, 128], f32, tag="ftmp")
                    nc.vector.tensor_scalar_mul(t[:], fl[:], w_b[:, l:l + 1])
                    nc.vector.tensor_tensor(out=acc[:], in0=acc[:], in1=t[:],
                                            op=mybir.AluOpType.add)
            nc.vector.tensor_scalar_add(acc[:], acc[:], cnnb_b[:, 0:1])
            nc.sync.dma_start(out=out_d[:, cols], in_=acc[:])

    nc.compile()
    _split_multiwaits(nc)
    return nc


def kernel(**inputs):
    global _last_exec_time_ns
    inputs = {k: np.asarray(v) for k, v in inputs.items()}

    meta = {}
    perview = {}
    for v in VIEWS:
        idx_arrs, sel_arrs, NLO, NHI = _prep_view(
            inputs[f"edges_{v}"].astype(np.int64), inputs[f"ew_{v}"])
        meta[v] = (NLO, NHI, int((NLO + NHI).sum()))
        perview[v] = (idx_arrs, sel_arrs)

    nc = _build(meta)

    x_pad = np.zeros((NPAD, FM), np.float32)
    x_pad[:N, :] = inputs["x_m"].astype(np.float32)
    x_pad = np.ascontiguousarray(x_pad).astype(bfnp)
    ident_np = np.eye(128, dtype=bfnp)
    # pad dst columns (node ids >= N, all on core 7) read relu(bias) into the
    # pooled sum; precompute the exact pollution per layer and subtract it.
    npad_cols = NPAD - N
    corr = np.array(
        [npad_cols * np.maximum(inputs[f"b_{nm}"].astype(np.float64), 0).sum()
         for nm in ["f1", "f2", "s1", "s2", "g1", "g2"]], np.float32)

    in_maps = []
    for c in range(NCORES):
        m = {
            "x_full": x_pad,
            "ident": ident_np,
            "fc1wT": inputs["fc1_w"].T.astype(np.float32).copy(),
            "fc1b": inputs["fc1_b"].astype(np.float32),
            "fc2wT": inputs["fc2_w"].T.astype(np.float32).copy(),
            "fc2b": inputs["fc2_b"].astype(np.float32),
            "cnnw": inputs["cnn_w"].astype(np.float32),
            "cnnb": inputs["cnn_b"].astype(np.float32),
            "corr": corr,
        }
        for nm in ["f1", "f2", "s1", "s2", "g1", "g2"]:
            m[f"W_{nm}"] = inputs[f"W_{nm}"].astype(bfnp)
            m[f"b_{nm}"] = inputs[f"b_{nm}"].astype(np.float32)
        for v in VIEWS:
            idx_arrs, sel_arrs = perview[v]
            m[f"idx_{v}"] = idx_arrs[c]
            m[f"sel_{v}"] = sel_arrs[c]
        in_maps.append(m)

    trace = os.environ.get("KERNEL_TRACE", "0") == "1"
    kw = {}
    if trace:
        td = os.environ.get("KERNEL_TRACE_DIR")
        if td:
            os.makedirs(td, exist_ok=True)
            kw["tmpdir"] = td
    res = run_bass_kernel_spmd(nc, in_maps, list(range(NCORES)), trace=trace, **kw)
    _last_exec_time_ns = res.exec_time_ns
    outT = np.concatenate([res.results[c]["out_slice"] for c in range(NCORES)], axis=1)
    return np.ascontiguousarray(outT.T[:N]).astype(np.float32)



# revision 16
# speedup vs baseline: 1.4271x; 1.0013x over previous
"""GCN message-passing kernel for 8 Trainium2 NeuronCores.

Strategy: shard destination nodes across cores (6272 rows/core). Each core
aggregates all edges targeting its rows by gathering source rows with
synchronous SWDGE dma_gather (1024 idxs/fire; >1024 hangs this ucode, and
every fire blocks the Pool engine ~8.5us regardless of mode, so sync issue
is optimal) and contracting each 128-edge chunk against a host-precomputed
one-hot selector (streamed from HBM) on the PE array. Layer-1 aggregations
use aggregate-then-transform (A@(xW) = (A@x)W): they gather raw x rows from
a replicated host-uploaded table and apply W on-PE afterwards, so no tab
phase or AllGather precedes them and the gather pipeline starts at t=0;
only the three layer-2 tables are computed+AllGathered (into 3 dedicated
DRAM buffers), overlapping the preceding aggregations. The aggregation runs
transposed (psum[feat, dst]) so bias+relu+row-sum fuse into one
Activation-engine op. SE attention + 1x1 conv are tiny and replicated; the
final output is produced transposed and fixed up on host.
"""
import os
import sys

sys.path.insert(0, "/opt/trn_rl_repo")

from contextlib import ExitStack

import ml_dtypes
import numpy as np

import concourse.bacc as bacc
import concourse.tile as tile
from concourse.tile import add_dep_helper
from concourse import bass_isa, mybir
from concourse.bass_utils import run_bass_kernel_spmd

N = 50000
FM = 128
E = 800000
NCORES = 8
NPOS = 49                  # 128-row tiles per core
RPC = NPOS * 128           # 6272 rows per core
NPAD = NCORES * RPC        # 50176
HALF = NPAD // 2           # 25088 (int16 gather index limit per table half)
SG = 4                     # positions per gather supergroup
VIEWS = ("f", "s", "g")
LAYERS = [("f", 1), ("s", 1), ("g", 1), ("f", 2), ("s", 2), ("g", 2)]

f32 = mybir.dt.float32
bf16 = mybir.dt.bfloat16
i16 = mybir.dt.int16
bfnp = ml_dtypes.bfloat16

_last_exec_time_ns = None


def _make_sgs(NLO, NHI, wmax=6, g=8):
    """Partition positions 0..NPOS-1 into consecutive groups (width<=wmax)
    minimizing total dma_gather fires sum(ceil(lo/g)+ceil(hi/g)); DP with
    width preference for ties (wider groups = fewer groups)."""
    PL = np.concatenate([[0], np.cumsum(NLO)])
    PH = np.concatenate([[0], np.cumsum(NHI)])
    INF = 1 << 30
    dp = [INF] * (NPOS + 1)
    back = [0] * (NPOS + 1)
    dp[0] = 0
    for i in range(1, NPOS + 1):
        for w in range(1, min(wmax, i) + 1):
            lo = PL[i] - PL[i - w]
            hi = PH[i] - PH[i - w]
            c = dp[i - w] + (lo + g - 1) // g + (hi + g - 1) // g
            if c < dp[i] or (c == dp[i] and w > back[i]):
                dp[i] = c
                back[i] = w
    sgs = []
    i = NPOS
    while i > 0:
        w = back[i]
        sgs.append(list(range(i - w, i)))
        i -= w
    return sgs[::-1]


def _split_multiwaits(nc):
    """This walrus build accepts only ONE sync-wait per instruction; split
    extras into preceding same-engine single-wait NoOps (sequencer executes
    waits in program order, so semantics are preserved)."""
    n = 0
    for fn in nc.m.functions:
        for bb in fn.blocks:
            newlist = []
            for inst in bb.instructions:
                si = inst.sync_info
                if si is not None and len(si.on_wait) > 1:
                    waits = list(si.on_wait)
                    for w in waits[:-1]:
                        nop = mybir.InstNoOp(name=f"WSPL-{nc.next_id()}", ins=[], outs=[])
                        nop.engine = inst.engine
                        nop.sync_info = mybir.SyncInfo(on_wait=[w], on_update=[])
                        newlist.append(nop)
                        n += 1
                    si.on_wait = [waits[-1]]
                newlist.append(inst)
            bb.instructions = newlist
    return n


def _prep_view(edges, ew):
    """Host edge preprocessing for one view: append self-loops, compute the
    symmetric GCN normalization, shard by destination across cores, group by
    (dst tile, src half), pad runs to 128-edge chunks (uniform across cores).

    Returns (idx_arrs, sel_arrs, NLO, NHI): per-core SWDGE index arrays and
    precomputed one-hot selector chunks ([128 edge-slot partitions, C*128
    dst columns], bf16, selector value = the edge's GCN norm weight)."""
    src = np.concatenate([edges[0], np.arange(N, dtype=np.int64)])
    dst = np.concatenate([edges[1], np.arange(N, dtype=np.int64)])
    w = np.concatenate([ew.astype(np.float64), np.ones(N)])
    deg = np.bincount(dst, weights=w, minlength=N)
    dis = 1.0 / np.sqrt(deg)
    norm = (dis[src] * w * dis[dst]).astype(np.float32)

    core = dst // RPC
    pos = (dst % RPC) // 128
    dstrel = (dst % 128).astype(np.int64)
    half = (src >= HALF).astype(np.int64)
    idx = (src - HALF * half).astype(np.int16)

    # counts[c, p, h]
    key = (core * NPOS + pos) * 2 + half
    counts = np.bincount(key, minlength=NCORES * NPOS * 2).reshape(NCORES, NPOS, 2)
    chunks = -(-counts // 128)  # ceil
    NLO = chunks[:, :, 0].max(axis=0)
    NHI = chunks[:, :, 1].max(axis=0)

    order = np.lexsort((half, pos, core))
    norm_s, dstrel_s, idx_s, key_s = (
        norm[order], dstrel[order], idx[order], key[order])
    starts = np.searchsorted(key_s, np.arange(NCORES * NPOS * 2))
    ends = np.searchsorted(key_s, np.arange(NCORES * NPOS * 2), side="right")

    C = int((NLO + NHI).sum())
    idx_arrs, sel_arrs = [], []
    sgs = _make_sgs(NLO, NHI)
    for c in range(NCORES):
        idx_a = np.zeros(C * 128, np.int16)
        dr_a = np.zeros(C * 128, np.int64)
        w_a = np.zeros(C * 128, np.float32)
        off = 0
        for sg in sgs:
            for h in range(2):
                for p in sg:
                    k = (c * NPOS + p) * 2 + h
                    s0, e0 = starts[k], ends[k]
                    n = e0 - s0
                    nch = (NLO if h == 0 else NHI)[p]
                    idx_a[off:off + n] = idx_s[s0:e0]
                    dr_a[off:off + n] = dstrel_s[s0:e0]
                    w_a[off:off + n] = norm_s[s0:e0]
                    off += nch * 128
        assert off == C * 128
        # device layouts
        idx_wrapped = np.tile(idx_a.reshape(-1, 16).T, (8, 1)).copy()  # [128, C*8]
        sel_flat = np.zeros((C * 128, 128), np.float32)
        sel_flat[np.arange(C * 128), dr_a] = w_a
        sel_dev = np.ascontiguousarray(
            sel_flat.reshape(C, 128, 128).transpose(1, 0, 2).reshape(128, C * 128)
        ).astype(bfnp)
        idx_arrs.append(idx_wrapped)
        sel_arrs.append(sel_dev)
    return idx_arrs, sel_arrs, NLO.astype(int), NHI.astype(int)


def _build(meta):
    """Build the SPMD program. meta[v] = (NLO, NHI, C) per view."""
    nc = bacc.Bacc("TRN2", target_bir_lowering=False, debug=False,
                   num_devices=NCORES)

    # ---- I/O ----
    xfull_in = nc.dram_tensor("x_full", [NPAD, FM], bf16, kind="ExternalInput").ap()
    W_in, b_in = {}, {}
    for nm in ["f1", "f2", "s1", "s2", "g1", "g2"]:
        W_in[nm] = nc.dram_tensor(f"W_{nm}", [FM, FM], bf16, kind="ExternalInput").ap()
        b_in[nm] = nc.dram_tensor(f"b_{nm}", [FM], f32, kind="ExternalInput").ap()
    idx_in, sel_in = {}, {}
    for v in VIEWS:
        C = meta[v][2]
        idx_in[v] = nc.dram_tensor(f"idx_{v}", [128, C * 8], i16, kind="ExternalInput").ap()
        sel_in[v] = nc.dram_tensor(f"sel_{v}", [128, C * 128], bf16, kind="ExternalInput").ap()
    ident_in = nc.dram_tensor("ident", [128, 128], bf16, kind="ExternalInput").ap()
    fc1wT_in = nc.dram_tensor("fc1wT", [6, 30], f32, kind="ExternalInput").ap()
    fc1b_in = nc.dram_tensor("fc1b", [30], f32, kind="ExternalInput").ap()
    fc2wT_in = nc.dram_tensor("fc2wT", [30, 6], f32, kind="ExternalInput").ap()
    fc2b_in = nc.dram_tensor("fc2b", [6], f32, kind="ExternalInput").ap()
    cnnw_in = nc.dram_tensor("cnnw", [6], f32, kind="ExternalInput").ap()
    cnnb_in = nc.dram_tensor("cnnb", [1], f32, kind="ExternalInput").ap()
    corr_in = nc.dram_tensor("corr", [6], f32, kind="ExternalInput").ap()
    out_d = nc.dram_tensor("out_slice", [FM, RPC], f32, kind="ExternalOutput").ap()

    dma_sem = nc.alloc_semaphore("gather_dma")

    with tile.TileContext(nc) as tc, ExitStack() as ctx:
        singles = ctx.enter_context(tc.tile_pool(name="singles", bufs=1))
        pool = ctx.enter_context(tc.tile_pool(name="pool", bufs=3))
        selp = ctx.enter_context(tc.tile_pool(name="selp", bufs=2))
        gpo = ctx.enter_context(tc.tile_pool(name="gpo", bufs=2))
        fpo = ctx.enter_context(tc.tile_pool(name="fpo", bufs=4))
        psA = ctx.enter_context(tc.tile_pool(name="psA", bufs=4, space="PSUM"))
        psB = ctx.enter_context(tc.tile_pool(name="psB", bufs=2, space="PSUM"))
        dram = ctx.enter_context(tc.tile_pool(name="dram", bufs=1, space="DRAM"))

        tables = [dram.tile([NPAD, FM], bf16, name=f"table{i}") for i in range(3)]
        tab_slices = [dram.tile([RPC, FM], bf16, name=f"tab_slice{i}") for i in range(2)]
        fT_sl = [dram.tile([128, RPC], bf16, name=f"fT_sl{i}") for i in range(6)]
        pool6_in = dram.tile([6, 1], f32, name="pool6_in")
        pool6_out = dram.tile([6, 1], f32, name="pool6_out")
        a_scr = dram.tile([6], f32, name="a_scr")

        # ---- constants ----
        ident = singles.tile([128, 128], bf16)
        nc.sync.dma_start(out=ident[:], in_=ident_in[:])
        identf = singles.tile([128, 128], f32)
        nc.vector.tensor_copy(out=identf[:], in_=ident[:])
        W_sb, bb_sb = {}, {}
        for nm in ["f1", "f2", "s1", "s2", "g1", "g2"]:
            W_sb[nm] = singles.tile([FM, FM], bf16, tag=f"W_{nm}", name=f"Wsb_{nm}")
            nc.sync.dma_start(out=W_sb[nm][:], in_=W_in[nm][:])
            bb_sb[nm] = singles.tile([FM, 1], f32, tag=f"bb_{nm}", name=f"bbsb_{nm}")
            nc.sync.dma_start(out=bb_sb[nm][:], in_=b_in[nm].unsqueeze(1))
        pooled_acc = singles.tile([128, 6], f32)
        nc.vector.memset(pooled_acc[:], 0.0)

        idx_sb = {}
        for v in VIEWS:
            C = meta[v][2]
            idx_sb[v] = singles.tile([128, C * 8], i16, tag=f"idx_{v}", name=f"idxsb_{v}")
            nc.sync.dma_start(out=idx_sb[v][:], in_=idx_in[v][:])

        # per-table-buffer state for manual collective/gather dep tracking
        # (custom-DMA APs over DRAM pool tiles are not reliably dep-tracked)
        tabst = [{"ag": None, "preps": []} for _ in range(3)]
        slice_ag = [None, None]   # last AllGather reading tab_slices[i]

        def tab_phase(src_kind, vsrc_l, Wn, ts):
            """tab_slices[ts] = cast_bf16(src @ W) for own rows.
            src 'x': xT input; src 'f': fT_sl[vsrc_l] (both [feat, node])."""
            war = slice_ag[ts]
            for p in range(NPOS):
                cols = slice(p * 128, (p + 1) * 128)
                t_fn = pool.tile([128, 128], bf16, tag="tabin")
                assert src_kind == "f"
                ld = nc.sync.dma_start(out=t_fn[:], in_=fT_sl[vsrc_l][:, cols])
                pm = psB.tile([128, 128], f32, tag="tabps")
                nc.tensor.matmul(pm[:], lhsT=W_sb[Wn][:], rhs=t_fn[:], start=True, stop=True)
                tmid = pool.tile([128, 128], f32, tag="tmid")
                nc.scalar.copy(out=tmid[:], in_=pm[:])
                ptr2 = psB.tile([128, 128], f32, tag="tabps2")
                nc.tensor.transpose(out=ptr2[:], in_=tmid[:], identity=identf[:])
                tb = pool.tile([128, 128], bf16, tag="tbf")
                nc.vector.tensor_copy(out=tb[:], in_=ptr2[:])
                st = nc.sync.dma_start(out=tab_slices[ts][p * 128:(p + 1) * 128, :], in_=tb[:])
                if war is not None:
                    add_dep_helper(st.ins, war.ins, reason="tab_slice WAR")
                yield st

        def allgather_table(tab_stores, ts, buf):
            ag = nc.gpsimd.collective_compute(
                "AllGather", mybir.AluOpType.bypass,
                replica_groups=[list(range(NCORES))],
                ins=[tab_slices[ts][:]], outs=[tables[buf][:]],
            )
            for st in tab_stores:
                add_dep_helper(ag.ins, st.ins, reason="tab_slice RAW")
            for g in tabst[buf]["preps"]:
                add_dep_helper(ag.ins, g.ins, reason="table WAR")
            tabst[buf] = {"ag": ag, "preps": []}
            slice_ag[ts] = ag

        def agg_phase(v, Wn, l_out, buf, from_x=False):
            NLO, NHI, C = meta[v]
            ag = None if from_x else tabst[buf]["ag"]
            sgs = _make_sgs(np.asarray(NLO), np.asarray(NHI))
            chunk_base = 0  # global chunk counter
            idx_col = 0     # column offset into idx_sb (units of 16 idxs)
            for sg in sgs:
                nlo = int(sum(NLO[p] for p in sg))
                nhi = int(sum(NHI[p] for p in sg))
                nch_sg = nlo + nhi
                # stream this supergroup's selector chunks (contiguous)
                selsb = selp.tile([128, max(nch_sg, 1) * 128], bf16, tag="sel")
                nc.sync.dma_start(
                    out=selsb[:],
                    in_=sel_in[v][:, chunk_base * 128:(chunk_base + max(nch_sg, 1)) * 128])
                glo = gpo.tile([128, max(nlo, 1), 128], bf16, tag="glo")
                ghi = gpo.tile([128, max(nhi, 1), 128], bf16, tag="ghi")
                GMAXC = int(os.environ.get("KERNEL_GMAXC", "8"))  # chunks per dma_gather (1024 idxs default; >=2048 hangs SWDGE)
                for half_i, (nh, gt, lohi) in enumerate(
                        (((nlo, glo, (0, HALF)), (nhi, ghi, (HALF, NPAD))))):
                    for g0 in range(0, nh, GMAXC):
                        gn = min(GMAXC, nh - g0)
                        src_tab = (xfull_in if from_x else tables[buf])
                        gi = nc.gpsimd.dma_gather(
                            out_ap=gt[:, g0:g0 + gn, :],
                            in_ap=src_tab[lohi[0]:lohi[1], :],
                            idxs_ap=idx_sb[v][:, idx_col:idx_col + gn * 8],
                            num_idxs=gn * 128, num_idxs_reg=gn * 128, elem_size=128,
                        )
                        if not from_x:
                            add_dep_helper(gi.ins, ag.ins, reason="table RAW")
                            tabst[buf]["preps"].append(gi)
                        idx_col += gn * 8
                # chunk order in sel array: [lo(p0)..lo(pk)] then [hi(p0)..hi(pk)]
                lo_off, off = {}, 0
                for p in sg:
                    lo_off[p] = off
                    off += int(NLO[p])
                hi_off, off = {}, 0
                for p in sg:
                    hi_off[p] = off
                    off += int(NHI[p])
                for p in sg:
                    ps = psA.tile([128, 128], f32, tag="agg")
                    nch = int(NLO[p] + NHI[p])
                    ci = 0
                    for k in range(int(NLO[p])):
                        cg = lo_off[p] + k  # sel col block within supergroup
                        nc.tensor.matmul(ps[:], lhsT=glo[:, lo_off[p] + k, :],
                                         rhs=selsb[:, cg * 128:(cg + 1) * 128],
                                         start=(ci == 0), stop=(ci == nch - 1))
                        ci += 1
                    for k in range(int(NHI[p])):
                        cg = nlo + hi_off[p] + k
                        nc.tensor.matmul(ps[:], lhsT=ghi[:, hi_off[p] + k, :],
                                         rhs=selsb[:, cg * 128:(cg + 1) * 128],
                                         start=(ci == 0), stop=(ci == nch - 1))
                        ci += 1
                    if from_x:
                        # aggregate-then-transform: psum holds (A@x)^T [fi,dst];
                        # apply W on-PE before bias+relu.
                        aggT = fpo.tile([128, 128], bf16, tag="aggT")
                        nc.scalar.copy(out=aggT[:], in_=ps[:])
                        ps2 = psB.tile([128, 128], f32, tag="tabps")
                        nc.tensor.matmul(ps2[:], lhsT=W_sb[Wn][:], rhs=aggT[:],
                                         start=True, stop=True)
                        ps = ps2
                    # postprocess: fT = relu(agg + b), fused row-sum for pooling
                    ft = fpo.tile([128, 128], bf16, tag="ftile")
                    racc = fpo.tile([128, 1], f32, tag="racc")
                    nc.scalar.activation(
                        out=ft[:], in_=ps[:], func=mybir.ActivationFunctionType.Relu,
                        bias=bb_sb[Wn][:, 0:1], accum_out=racc[:])
                    nc.vector.tensor_tensor(out=pooled_acc[:, l_out:l_out + 1],
                                            in0=pooled_acc[:, l_out:l_out + 1],
                                            in1=racc[:], op=mybir.AluOpType.add)
                    nc.scalar.dma_start(out=fT_sl[l_out][:, p * 128:(p + 1) * 128],
                                        in_=ft[:])
                chunk_base += nch_sg

        scope = os.environ.get("KERNEL_SCOPE", "full")
        if scope == "full":
            # schedule: tab f1, AG f1, tab s1, AG s1, agg f1, tab g1, AG g1,
            # agg s1, tab f2, AG f2, agg g1, ... so each AllGather overlaps
            # the previous layer's aggregation (ping-pong table buffers).
            plans = []
            for i, (v, ln) in enumerate(LAYERS):
                nm = f"{v}{ln}"
                src = ("x", None) if ln == 1 else ("f", 2 * VIEWS.index(v))
                plans.append({"v": v, "nm": nm, "src": src, "l_out": 2 * VIEWS.index(v) + ln - 1,
                              "buf": VIEWS.index(v), "ts": i % 2})

            def do_tab(i):
                p = plans[i]
                stores = list(tab_phase(p["src"][0], p["src"][1], p["nm"], p["ts"]))
                allgather_table(stores, p["ts"], p["buf"])

            def do_agg(i, from_x=False):
                p = plans[i]
                agg_phase(p["v"], p["nm"], p["l_out"], p["buf"], from_x=from_x)

            # layer-1 aggregations gather raw x rows from the replicated input
            # table (no tab phase, no AllGather) and post-multiply by W; only
            # the second layers need table AllGathers.
            do_agg(0, from_x=True)
            do_tab(3)
            do_agg(1, from_x=True)
            do_tab(4)
            do_agg(2, from_x=True)
            do_tab(5)
            do_agg(3)
            do_agg(4)
            do_agg(5)
        else:
            plans = [{"src": ("x", None), "nm": "f1", "ts": 0, "buf": 0, "v": "f",
                      "l_out": 0}]
            if scope in ("agg1", "f1out"):
                agg_phase("f", "f1", 0, 0, from_x=True)

        # ---- pooled -> SE attention scalars ----
        pool_red = singles.tile([128, 6], f32)
        nc.gpsimd.partition_all_reduce(pool_red[:], pooled_acc[:], 128,
                                       bass_isa.ReduceOp.add)
        nc.sync.dma_start(out=pool6_in[:], in_=pool_red[0:1, 0:6])
        nc.gpsimd.collective_compute(
            "AllReduce", mybir.AluOpType.add,
            replica_groups=[list(range(NCORES))],
            ins=[pool6_in[:]], outs=[pool6_out[:]],
        )
        pvec2 = singles.tile([6, 1], f32)
        nc.sync.dma_start(out=pvec2[:], in_=pool6_out[:])
        corr_sb = singles.tile([6, 1], f32)
        nc.sync.dma_start(out=corr_sb[:], in_=corr_in.unsqueeze(1))
        # remove pad-column relu(bias) pollution, then mean
        nc.vector.tensor_tensor(out=pvec2[:], in0=pvec2[:], in1=corr_sb[:],
                                op=mybir.AluOpType.subtract)
        nc.vector.tensor_scalar_mul(pvec2[:], pvec2[:], 1.0 / (N * FM))
        fc1wT = singles.tile([6, 30], f32)
        nc.sync.dma_start(out=fc1wT[:], in_=fc1wT_in[:])
        fc1b = singles.tile([30, 1], f32)
        nc.sync.dma_start(out=fc1b[:], in_=fc1b_in.unsqueeze(1))
        fc2wT = singles.tile([30, 6], f32)
        nc.sync.dma_start(out=fc2wT[:], in_=fc2wT_in[:])
        fc2b = singles.tile([6, 1], f32)
        nc.sync.dma_start(out=fc2b[:], in_=fc2b_in.unsqueeze(1))
        pz1 = psB.tile([30, 1], f32, tag="tabps")
        nc.tensor.matmul(pz1[:], lhsT=fc1wT[:], rhs=pvec2[:], start=True, stop=True)
        z1 = singles.tile([30, 1], f32)
        nc.vector.tensor_tensor(out=z1[:], in0=pz1[:], in1=fc1b[:], op=mybir.AluOpType.add)
        nc.vector.tensor_scalar_max(z1[:], z1[:], 0.0)
        pz2 = psB.tile([6, 1], f32, tag="tabps")
        nc.tensor.matmul(pz2[:], lhsT=fc2wT[:], rhs=z1[:], start=True, stop=True)
        z2 = singles.tile([6, 1], f32)
        nc.vector.tensor_tensor(out=z2[:], in0=pz2[:], in1=fc2b[:], op=mybir.AluOpType.add)
        av = singles.tile([6, 1], f32)
        nc.scalar.activation(out=av[:], in_=z2[:], func=mybir.ActivationFunctionType.Sigmoid)
        cnnw6 = singles.tile([6, 1], f32)
        nc.sync.dma_start(out=cnnw6[:], in_=cnnw_in.unsqueeze(1))
        # a >= 0 (sigmoid) and fT >= 0 (post-relu), so relu(a*fT) == a*fT and
        # the combine is linear: out = sum_l (a_l*cnnw_l)*fT_l + cnnb.
        wv = singles.tile([6, 1], f32)
        nc.vector.tensor_tensor(out=wv[:], in0=av[:], in1=cnnw6[:],
                                op=mybir.AluOpType.mult)
        nc.sync.dma_start(out=a_scr[:], in_=wv[:, 0])
        w_b = singles.tile([128, 6], f32)
        nc.gpsimd.dma_start(out=w_b[:], in_=a_scr[:].partition_broadcast(128))
        cnnb_b = singles.tile([128, 1], f32)
        nc.gpsimd.dma_start(out=cnnb_b[:], in_=cnnb_in.partition_broadcast(128))

        # ---- final combine: outT = sum_l cnnw_l * relu(a_l * fT_l) + cnn_b ----
        if scope == "f1out":
            for p in range(NPOS):
                cols = slice(p * 128, (p + 1) * 128)
                fl0 = fpo.tile([128, 128], bf16, tag="fin", name=f"fl0_{p}")
                nc.sync.dma_start(out=fl0[:], in_=fT_sl[0][:, cols])
                fo = fpo.tile([128, 128], f32, tag="ftmp", name=f"fo_{p}")
                nc.vector.tensor_copy(out=fo[:], in_=fl0[:])
                nc.sync.dma_start(out=out_d[:, cols], in_=fo[:])
        nlayers = 6 if scope == "full" else 1
        for p in range(NPOS) if scope != "f1out" else []:
            cols = slice(p * 128, (p + 1) * 128)
            acc = fpo.tile([128, 128], f32, tag="facc")
            for l in range(nlayers):
                fl = fpo.tile([128, 128], bf16, tag="fin")
                nc.sync.dma_start(out=fl[:], in_=fT_sl[l][:, cols])
                if l == 0:
                    nc.vector.tensor_scalar_mul(acc[:], fl[:], w_b[:, 0:1])
                else:
                    t = fpo.tile([128, 128], f32, tag="ftmp")
                    nc.vector.tensor_scalar_mul(t[:], fl[:], w_b[:, l:l + 1])
                    nc.vector.tensor_tensor(out=acc[:], in0=acc[:], in1=t[:],
                                            op=mybir.AluOpType.add)
            nc.vector.tensor_scalar_add(acc[:], acc[:], cnnb_b[:, 0:1])
            nc.sync.dma_start(out=out_d[:, cols], in_=acc[:])

    nc.compile()
    _split_multiwaits(nc)
    return nc


def kernel(**inputs):
    global _last_exec_time_ns
    inputs = {k: np.asarray(v) for k, v in inputs.items()}

    meta = {}
    perview = {}
    for v in VIEWS:
        idx_arrs, sel_arrs, NLO, NHI = _prep_view(
            inputs[f"edges_{v}"].astype(np.int64), inputs[f"ew_{v}"])
        meta[v] = (NLO, NHI, int((NLO + NHI).sum()))
        perview[v] = (idx_arrs, sel_arrs)

    nc = _build(meta)

    x_pad = np.zeros((NPAD, FM), np.float32)
    x_pad[:N, :] = inputs["x_m"].astype(np.float32)
    x_pad = np.ascontiguousarray(x_pad).astype(bfnp)
    ident_np = np.eye(128, dtype=bfnp)
    # pad dst columns (node ids >= N, all on core 7) read relu(bias) into the
    # pooled sum; precompute the exact pollution per layer and subtract it.
    npad_cols = NPAD - N
    corr = np.array(
        [npad_cols * np.maximum(inputs[f"b_{nm}"].astype(np.float64), 0).sum()
         for nm in ["f1", "f2", "s1", "s2", "g1", "g2"]], np.float32)

    in_maps = []
    for c in range(NCORES):
        m = {
            "x_full": x_pad,
            "ident": ident_np,
            "fc1wT": inputs["fc1_w"].T.astype(np.float32).copy(),
            "fc1b": inputs["fc1_b"].astype(np.float32),
            "fc2wT": inputs["fc2_w"].T.astype(np.float32).copy(),
            "fc2b": inputs["fc2_b"].astype(np.float32),
            "cnnw": inputs["cnn_w"].astype(np.float32),
            "cnnb": inputs["cnn_b"].astype(np.float32),
            "corr": corr,
        }
        for nm in ["f1", "f2", "s1", "s2", "g1", "g2"]:
            m[f"W_{nm}"] = inputs[f"W_{nm}"].astype(bfnp)
            m[f"b_{nm}"] = inputs[f"b_{nm}"].astype(np.float32)
        for v in VIEWS:
            idx_arrs, sel_arrs = perview[v]
            m[f"idx_{v}"] = idx_arrs[c]
            m[f"sel_{v}"] = sel_arrs[c]
        in_maps.append(m)

    trace = os.environ.get("KERNEL_TRACE", "0") == "1"
    kw = {}
    if trace:
        td = os.environ.get("KERNEL_TRACE_DIR")
        if td:
            os.makedirs(td, exist_ok=True)
            kw["tmpdir"] = td
    res = run_bass_kernel_spmd(nc, in_maps, list(range(NCORES)), trace=trace, **kw)
    _last_exec_time_ns = res.exec_time_ns
    outT = np.concatenate([res.results[c]["out_slice"] for c in range(NCORES)], axis=1)
    return np.ascontiguousarray(outT.T[:N]).astype(np.float32)

